# revision 1
# baseline (speedup 1.0000x reference)
"""Trainium2 Bass kernel for masked single-head attention.

Reference computation (per batch b):
    Q = q_hidden[b] @ Wq + bq            # [S, D]
    K = k_hidden[b] @ Wk + bk            # [S, D]
    V = v_hidden[b] @ Wv + bv            # [S, D]
    S_qk = (Q @ K.T) / sqrt(D)           # [S, S]
    S_qk = where(mask[b]==0, -1e9, S_qk)
    out[b] = softmax(S_qk, -1) @ V       # [S, D]

Sharding: data-parallel over batch, one batch per NeuronCore (B == 8 cores).
No collectives.

Device-side dataflow (per core, S=2048, HID=1024, D=64):
  - host ships transposed hiddens qT/kT/vT [HID, S] (fp16) and the mask
    transposed as (m-1) in fp8 {-1,0}; Wq/bq pre-scaled by 1/sqrt(D).
  - DMAs are c-chunk-major and ordered q(half0), k, q(half1), v, mask so the
    projections and then the score matmuls can chase the arriving data.
  - projections on PE, column-packed: two 512-wide s-chunks go to array
    column groups 0-63 / 64-127 concurrently (PSUM partitions 0-63/64-127).
  - Q^T/K^T live in [128, S] tiles with rows 64-127 duplicating rows 0-63
    (SBUF->SBUF DMA) so score matmuls can row-pack: two k-tiles run
    concurrently on array row groups 0-63 / 64-127 (contraction dim is 64).
  - scores^T for a k-tile pair land in one [128, 1024] PSUM tile
    ([ktA q-512 | ktB q-512]) via float32r matmuls; the mask is applied in
    the same accumulation as an extra matmul (48*I_fp8).T @ (m-1)_fp8.
  - one exp on ScalarE per pair covers [128, 1024]; masked entries become
    exp(s-48) ~ 1e-19*exp(s), which vanishes against the row sum.
  - out^T[65, q] += [V|1].T @ P^T accumulated over k: rows 0..63 numerator,
    row 64 the softmax denominator (ones column appended to V).
  - reciprocal of the denominator row, PE-transpose of [65,128] slices back
    to [128,65], multiply by the per-partition reciprocal, DMA out [q,64].
"""

import os
import numpy as np
import ml_dtypes

import concourse.bass as bass
import concourse.tile as tile
from concourse import bacc
from concourse import mybir
from concourse.bass_utils import run_bass_kernel_spmd

B, S, HID, D = 8, 2048, 1024, 64
NCORES = 8
HCH = HID // 128          # 8 hidden chunks
KT_TILES = S // 128       # 16 k tiles
NQ = 512                  # q chunk width for the attention inner loop
QCH = S // NQ             # 4
MASK_C = 48.0             # mask offset constant (exactly representable in e4m3)

F32 = mybir.dt.float32
F32R = mybir.dt.float32r
FP8 = mybir.dt.float8e4

_HID_DT_NAME = os.environ.get("ATT_HID_DT", "f16")
HID_DT = mybir.dt.float16 if _HID_DT_NAME == "f16" else F32
HID_NP = np.float16 if _HID_DT_NAME == "f16" else np.float32
FP8_NP = ml_dtypes.float8_e4m3

LAST_EXEC_TIME_NS = None
_CACHED = {}


def _build_program(with_qk_bias=False, reps=1, ablate='full'):
    nc = bacc.Bacc("TRN2", target_bir_lowering=False, debug=False,
                   num_swdge_queues=4)

    qT_d = nc.dram_tensor("qT", [HID, S], HID_DT, kind="ExternalInput").ap()
    kT_d = nc.dram_tensor("kT", [HID, S], HID_DT, kind="ExternalInput").ap()
    vT_d = nc.dram_tensor("vT", [HID, S], HID_DT, kind="ExternalInput").ap()
    maskT_d = nc.dram_tensor("maskT", [S, S], FP8, kind="ExternalInput").ap()
    wq_d = nc.dram_tensor("wq", [HID, D], HID_DT, kind="ExternalInput").ap()
    wk_d = nc.dram_tensor("wk", [HID, D], HID_DT, kind="ExternalInput").ap()
    wv_d = nc.dram_tensor("wv", [HID, D], HID_DT, kind="ExternalInput").ap()
    if with_qk_bias:
        bq_d = nc.dram_tensor("bq", [D], F32, kind="ExternalInput").ap()
        bk_d = nc.dram_tensor("bk", [D], F32, kind="ExternalInput").ap()
    idm_d = nc.dram_tensor("idm", [128, 128], FP8, kind="ExternalInput").ap()
    idf_d = nc.dram_tensor("idf", [128, 128], F32, kind="ExternalInput").ap()
    out_d = nc.dram_tensor("out", [S, D], F32, kind="ExternalOutput").ap()

    ExpF = mybir.ActivationFunctionType.Exp

    def _body(tc):
        with tc.tile_pool(name="const", bufs=1) as const:
            w_q = const.tile([128, HCH, D], HID_DT, name="w_q")
            w_k = const.tile([128, HCH, D], HID_DT, name="w_k")
            w_v = const.tile([128, HCH, D], HID_DT, name="w_v")
            nc.sync.dma_start(w_q, wq_d.rearrange("(o p) d -> p o d", p=128))
            nc.sync.dma_start(w_k, wk_d.rearrange("(o p) d -> p o d", p=128))
            nc.sync.dma_start(w_v, wv_d.rearrange("(o p) d -> p o d", p=128))
            if with_qk_bias:
                b_q = const.tile([128, 1], F32, name="b_q")
                b_k = const.tile([128, 1], F32, name="b_k")
                nc.sync.dma_start(b_q[0:D, :], bq_d.unsqueeze(1))
                nc.sync.dma_start(b_q[64:64 + D, :], bq_d.unsqueeze(1))
                nc.sync.dma_start(b_k[0:D, :], bk_d.unsqueeze(1))
                nc.sync.dma_start(b_k[64:64 + D, :], bk_d.unsqueeze(1))
            else:
                b_q = b_k = None
            idm = const.tile([128, 128], FP8, name="idm")
            idf = const.tile([128, 128], F32, name="idf")
            nc.sync.dma_start(idm, idm_d)
            nc.sync.dma_start(idf, idf_d)
            idf16 = const.tile([128, 128], HID_DT, name="idf16")
            nc.vector.tensor_copy(idf16, idf)

            masksb = const.tile([128, KT_TILES, S], FP8, name="masksb")
            qh = const.tile([128, HCH, S], HID_DT, name="qh")
            kh = const.tile([128, HCH, S], HID_DT, name="kh")
            vh = const.tile([128, HCH, S], HID_DT, name="vh")

            # DMA issue order matches the consumption order of the staged
            # compute below: q chunk0 -> all of k -> all masks -> q chunk1 ->
            # all of v -> q chunk2 -> q chunk3.  The first exp needs only
            # ~4 MB; later q chunks gate only their own attention stage.
            def dma_hid(t, d, c0, c1, eng):
                csl = slice(c0, c1)
                for h in range(HCH):
                    eng.dma_start(t[:, h, csl],
                                  d[h * 128:(h + 1) * 128, csl])

            dma_mode = os.environ.get("ATT_DMA_MODE", "spread")
            if dma_mode == "orig":
                for c in range(QCH):
                    dma_hid(qh, qT_d, c * NQ, (c + 1) * NQ, nc.sync)
                dma_hid(kh, kT_d, 0, S // 2, nc.sync)
                dma_hid(kh, kT_d, S // 2, S, nc.sync)
                for kt in range(KT_TILES):
                    nc.sync.dma_start(masksb[:, kt, :],
                                      maskT_d[kt * 128:(kt + 1) * 128, :])
                dma_hid(vh, vT_d, 0, S // 2, nc.sync)
                dma_hid(vh, vT_d, S // 2, S, nc.sync)
            else:
                # spread across the three DMA-capable engine queues so the
                # per-DMA issue overheads overlap: q on SP, k on ACT,
                # masks on gpsimd, v split between SP and ACT.
                coarse = os.environ.get("ATT_DMA_COARSE", "0") == "1"
                if coarse:
                    dma_hid(qh, qT_d, 0, S, nc.sync)
                    dma_hid(kh, kT_d, 0, S, nc.scalar)
                    for kt in range(0, KT_TILES, 2):
                        nc.gpsimd.dma_start(
                            masksb[:, kt:kt + 2, :],
                            maskT_d[kt * 128:(kt + 2) * 128, :].rearrange(
                                "(t p) s -> p t s", p=128))
                    dma_hid(vh, vT_d, 0, 1024, nc.sync)
                    dma_hid(vh, vT_d, 1024, 2048, nc.scalar)
                else:
                    for c in range(2):
                        dma_hid(qh, qT_d, c * 1024, (c + 1) * 1024, nc.sync)
                    for c in range(2):
                        dma_hid(kh, kT_d, c * 1024, (c + 1) * 1024, nc.scalar)
                    for kt in range(KT_TILES):
                        nc.gpsimd.dma_start(masksb[:, kt, :],
                                            maskT_d[kt * 128:(kt + 1) * 128, :])
                    dma_hid(vh, vT_d, 0, 1024, nc.sync)
                    dma_hid(vh, vT_d, 1024, 2048, nc.scalar)

            if ablate == 'dma':
                return
            # QT/KT: rows 64-127 duplicate rows 0-63 (for row-packed scores).
            sc_dt = (mybir.dt.float16 if os.environ.get("ATT_SC_DT", "f16")
                     == "f16" else F32R)
            QT = const.tile([128, S], sc_dt, name="QT")
            KT = const.tile([128, S], sc_dt, name="KT")
            VT = const.tile([128, S], HID_DT, name="VT")
            Vt = const.tile([128, KT_TILES, D + 1], HID_DT, name="Vt")

            with tc.tile_pool(name="stp", bufs=2, space="PSUM") as stp, \
                 tc.tile_pool(name="ntp", bufs=2, space="PSUM") as ntp, \
                 tc.tile_pool(name="ptp", bufs=24) as ptp, \
                 tc.tile_pool(name="nsb", bufs=2) as nsb:
                NPAIR = KT_TILES // 2
                ones_ap = nc.const_aps.tensor(1.0, (128, 1))

                def q_proj(c):
                    # one 512-wide q chunk, computed into BOTH array column
                    # groups concurrently so QT rows 0-63 and 64-127 both get
                    # the data without any cross-partition copy.
                    cs = slice(c * NQ, (c + 1) * NQ)
                    prja = stp.tile([128, NQ], F32, name="prja", tag="prj",
                                    bufs=2)
                    prjb = stp.tile([128, NQ], F32, name="prjb", tag="prj",
                                    bufs=2)
                    for h in range(HCH):
                        nc.tensor.matmul(
                            prja[0:D, :], lhsT=w_q[:, h, :],
                            rhs=qh[:, h, cs],
                            start=(h == 0), stop=(h == HCH - 1))
                        nc.tensor.matmul(
                            prjb[64:64 + D, :], lhsT=w_q[:, h, :],
                            rhs=qh[:, h, cs],
                            start=(h == 0), stop=(h == HCH - 1))
                    nc.vector.tensor_copy(QT[0:D, cs], prja[0:D, :])
                    nc.vector.tensor_copy(QT[64:64 + D, cs],
                                          prjb[64:64 + D, :])
                    if b_q is not None:
                        nc.vector.tensor_scalar_add(
                            QT[0:D, cs], QT[0:D, cs], b_q[0:D, :])
                        nc.vector.tensor_scalar_add(
                            QT[64:64 + D, cs], QT[64:64 + D, cs],
                            b_q[64:64 + D, :])

                def kv_proj(hid_t, w_t, b_t, dest):
                    # column-packed pairs of 512-chunks
                    for cp in range(2):
                        ca = slice((2 * cp) * 512, (2 * cp + 1) * 512)
                        cb = slice((2 * cp + 1) * 512, (2 * cp + 2) * 512)
                        prja = stp.tile([128, 512], F32, name="prja",
                                        tag="prj", bufs=2)
                        prjb = stp.tile([128, 512], F32, name="prjb",
                                        tag="prj", bufs=2)
                        for h in range(HCH):
                            nc.tensor.matmul(
                                prja[0:D, :], lhsT=w_t[:, h, :],
                                rhs=hid_t[:, h, ca],
                                start=(h == 0), stop=(h == HCH - 1))
                            nc.tensor.matmul(
                                prjb[64:64 + D, :], lhsT=w_t[:, h, :],
                                rhs=hid_t[:, h, cb],
                                start=(h == 0), stop=(h == HCH - 1))
                        nc.vector.tensor_copy(dest[0:D, ca], prja[0:D, :])
                        nc.vector.tensor_copy(dest[64:64 + D, cb],
                                              prjb[64:64 + D, :])
                        if b_t is not None:
                            nc.vector.tensor_scalar_add(
                                dest[0:D, ca], dest[0:D, ca], b_t[0:D, :])
                            nc.vector.tensor_scalar_add(
                                dest[64:64 + D, cb], dest[64:64 + D, cb],
                                b_t[64:64 + D, :])

                def v_finish():
                    # V^T -> V tiles with ones column; odd 512-chunks of VT
                    # live on rows 64-127 (column packing), so use the
                    # identity's matching diagonal block.
                    for kt in range(KT_TILES):
                        rb = 0 if (kt // 4) % 2 == 0 else 64
                        vtr = ntp.tile([128, D], HID_DT, name="vtr",
                                       tag="tr")
                        nc.tensor.transpose(
                            vtr, VT[rb:rb + D, kt * 128:(kt + 1) * 128],
                            idf16[rb:rb + D, rb:rb + D])
                        nc.vector.tensor_copy(Vt[:, kt, :D], vtr)
                        nc.vector.tensor_copy(Vt[:, kt, D:D + 1], ones_ap)

                def sc_exp(qc, p, mode='all'):
                    # row-packed score pair + mask accumulate + exp.
                    # pair (kt, kt+4): kta lives in an even 512-chunk of KT
                    # (rows 0-63), ktb = kta+4 in the next odd chunk, which
                    # column packing left on rows 64-127 - no KT duplication.
                    q0 = qc * NQ
                    qsl = slice(q0, q0 + NQ)
                    g, i = divmod(p, 4)
                    kta, ktb = 8 * g + i, 8 * g + i + 4
                    sa = slice(kta * 128, kta * 128 + 128)
                    sb = slice(ktb * 128, ktb * 128 + 128)
                    st = stp.tile([128, 2 * NQ], F32, name="st", tag="st")
                    mm_stop = mode not in ('all', 'scmask')
                    if mode != 'maskonly':
                        nc.tensor.matmul(
                            st[:, 0:NQ], lhsT=KT[0:D, sa], rhs=QT[0:D, qsl],
                            start=True, stop=mm_stop)
                        nc.tensor.matmul(
                            st[:, NQ:2 * NQ], lhsT=KT[64:64 + D, sb],
                            rhs=QT[64:64 + D, qsl],
                            start=True, stop=mm_stop)
                    if mode in ('all', 'scmask', 'maskonly'):
                        mst = (mode == 'maskonly')
                        nc.tensor.matmul(
                            st[:, 0:NQ], lhsT=idm, rhs=masksb[:, kta, qsl],
                            start=mst, stop=True)
                        nc.tensor.matmul(
                            st[:, NQ:2 * NQ], lhsT=idm,
                            rhs=masksb[:, ktb, qsl],
                            start=mst, stop=True)
                    pt = ptp.tile([128, 2 * NQ], HID_DT, name="pt", tag="pt")
                    if mode in ('all', 'scexp'):
                        nc.scalar.activation(pt, st, ExpF)
                    else:
                        nc.vector.tensor_copy(pt[0:1, 0:16], st[0:1, 0:16])
                    return pt

                def av(outT, p, pt, npair):
                    g, i = divmod(p, 4)
                    kta, ktb = 8 * g + i, 8 * g + i + 4
                    nc.tensor.matmul(
                        outT, lhsT=Vt[:, kta, :], rhs=pt[:, 0:NQ],
                        start=(p == 0), stop=False)
                    nc.tensor.matmul(
                        outT, lhsT=Vt[:, ktb, :], rhs=pt[:, NQ:2 * NQ],
                        start=False, stop=(p == npair - 1))

                def norm(qc, outT):
                    q0 = qc * NQ
                    outT_sb = nsb.tile([D + 1, NQ], F32, name="outT_sb",
                                       tag="outT_sb")
                    nc.vector.tensor_copy(outT_sb, outT)
                    nc.vector.reciprocal(outT_sb[D:D + 1, :],
                                         outT_sb[D:D + 1, :])
                    o_big = nsb.tile([128, NQ // 128, D], F32, name="o_big",
                                     tag="o_big")
                    for i in range(NQ // 128):
                        tr = ntp.tile([128, D + 1], F32, name="tr", tag="tr")
                        nc.tensor.transpose(
                            tr, outT_sb[:, i * 128:(i + 1) * 128],
                            idf[:D + 1, :D + 1])
                        tr_sb = nsb.tile([128, D + 1], F32, name="tr_sb",
                                         tag="tr_sb")
                        nc.vector.tensor_copy(tr_sb, tr)
                        nc.vector.tensor_scalar_mul(
                            o_big[:, i, :], tr_sb[:, :D], tr_sb[:, D:D + 1])
                    nc.sync.dma_start(
                        out_d[q0:q0 + NQ, :].rearrange("(t p) d -> p t d",
                                                       p=128), o_big)

                # ---- staged emission (PE stream order == data arrival) ----
                for c in range(QCH):
                    q_proj(c)
                kv_proj(kh, w_k, b_k, KT)
                if ablate == 'proj':
                    kv_proj(vh, w_v, None, VT)
                    v_finish()
                    return
                pts = {}
                sc_mode = ablate if ablate in ('sc', 'scmask', 'scexp', 'maskonly') \
                    else 'all'
                for qc in range(QCH):
                    for p in range(NPAIR):
                        pts[(qc, p)] = sc_exp(qc, p, sc_mode)
                kv_proj(vh, w_v, None, VT)
                v_finish()
                if ablate in ('noav', 'sc', 'scmask', 'scexp', 'maskonly'):
                    return
                for qc in range(QCH):
                    outT = stp.tile([D + 1, NQ], F32, name="outT",
                                    tag="prj", bufs=2)
                    for p in range(NPAIR):
                        av(outT, p, pts[(qc, p)], NPAIR)
                    if ablate != 'nonorm':
                        norm(qc, outT)
                    else:
                        nc.vector.tensor_copy(
                            nsb.tile([D + 1, NQ], F32, name="outT_sb",
                                     tag="outT_sb"), outT)

    with tile.TileContext(nc) as tc:
        if reps > 1:
            with tc.For_i(0, reps, 1):
                _body(tc)
        else:
            _body(tc)

    nc.compile()
    return nc


def _prep_inputs(q_hidden_inputs, k_hidden_inputs, v_hidden_inputs, mask,
                 Wq, bq, Wk, bk, Wv, bv):
    scale = np.float32(1.0 / np.sqrt(np.float32(D)))
    wq = (np.asarray(Wq, np.float32) * scale).astype(HID_NP)
    wk = np.asarray(Wk, np.float32).astype(HID_NP)
    wv = np.asarray(Wv, np.float32).astype(HID_NP)
    bqs = (np.asarray(bq, np.float32) * scale)
    bks = np.asarray(bk, np.float32)
    with_qk_bias = bool(np.any(bqs != 0) or np.any(bks != 0))
    idm = (np.eye(128, dtype=np.float32) * MASK_C).astype(FP8_NP)
    idf = np.eye(128, dtype=np.float32)

    q = np.asarray(q_hidden_inputs, np.float32)
    k = np.asarray(k_hidden_inputs, np.float32)
    v = np.asarray(v_hidden_inputs, np.float32)
    m = np.asarray(mask)

    in_maps = []
    for b in range(B):
        im = {
            "qT": np.ascontiguousarray(q[b].T).astype(HID_NP),
            "kT": np.ascontiguousarray(k[b].T).astype(HID_NP),
            "vT": np.ascontiguousarray(v[b].T).astype(HID_NP),
            "maskT": (np.ascontiguousarray(m[b].T) - np.int32(1)).astype(
                np.float32).astype(FP8_NP),
            "wq": wq, "wk": wk, "wv": wv,
            "idm": idm, "idf": idf,
        }
        if with_qk_bias:
            im["bq"] = bqs
            im["bk"] = bks
        in_maps.append(im)
    return in_maps, with_qk_bias


def kernel(q_hidden_inputs, k_hidden_inputs, v_hidden_inputs, mask,
           Wq, bq, Wk, bk, Wv, bv, trace=False):
    global LAST_EXEC_TIME_NS
    in_maps, with_qk_bias = _prep_inputs(
        q_hidden_inputs, k_hidden_inputs, v_hidden_inputs,
        mask, Wq, bq, Wk, bk, Wv, bv)
    key = ("nc", with_qk_bias)
    if key not in _CACHED:
        _CACHED[key] = _build_program(with_qk_bias)
    nc = _CACHED[key]

    res = run_bass_kernel_spmd(nc, in_maps, list(range(NCORES)), trace=trace)
    LAST_EXEC_TIME_NS = res.exec_time_ns
    out = np.stack([res.results[b]["out"] for b in range(B)], axis=0)
    # bv folds into the output exactly: softmax rows sum to 1, so
    # attn @ (V + 1 bv^T) = attn @ V + bv.
    out = out + np.asarray(bv, np.float32)[None, None, :]
    return out



# revision 5
# speedup vs baseline: 1.0183x; 1.0183x over previous
"""Trainium2 Bass kernel for masked single-head attention.

Reference computation (per batch b):
    Q = q_hidden[b] @ Wq + bq            # [S, D]
    K = k_hidden[b] @ Wk + bk            # [S, D]
    V = v_hidden[b] @ Wv + bv            # [S, D]
    S_qk = (Q @ K.T) / sqrt(D)           # [S, S]
    S_qk = where(mask[b]==0, -1e9, S_qk)
    out[b] = softmax(S_qk, -1) @ V       # [S, D]

Sharding: data-parallel over batch, one batch per NeuronCore (B == 8 cores).
No collectives.

Device-side dataflow (per core, S=2048, HID=1024, D=64).  The kernel is
purely TensorE-row-stream bound, so the design minimizes total matmul
moving-rows (cost per matmul = out free size, regardless of contraction
or partition count):
  - host ships transposed hiddens qT/kT/vT [HID, S] fp16, the mask as
    48*(mT-1) in fp8 {0,-48}, and doubled weights [W|W] [HID, 2D] so one
    projection pass writes both PSUM partition groups 0-63 / 64-127
    (the row-duplicated layout the row-packed score matmuls need) in a
    single N-row stream.  Wq is pre-scaled by 1/sqrt(D).
  - projections on PE: per 512-col chunk, 8 contraction chunks,
    lhsT=[W|W][128,128] -> PSUM [128,512]; one DVE cast to f16
    QT/KT/VT [128, S] (rows 64-127 duplicate 0-63).
  - scores^T for k-tile pair (2j, 2j+1) land in one [128, 1024] PSUM
    tile via two row-packed f16 matmuls (contraction D=64 on partitions
    0-63 / 64-127).
  - mask is applied by DVE: pre = st + 48*(mT-1)  (PSUM f32 + fp8 ->
    f16 SBUF).  This frees the PSUM tile at DVE speed and costs the PE
    nothing.  exp runs on ScalarE (ACT) decoupled from the PE stream:
    masked entries become exp(s-48) which underflows f16 to exact 0.
  - out^T[65, q] += [V|1].T @ P^T accumulated over k-tiles: rows 0..63
    numerator, row 64 the softmax denominator.
  - norm: cast outT to f16, PE-transpose [65,128] slices back to
    [128,65], reciprocal of the [128,1] denominator column, multiply,
    DMA out [q, 64] as f32.
  - DMA: few large descriptors; q on SP (first), k/v on ACT queue
    (early, before exp starts), mask on gpsimd.  qh/kh live in a scoped
    pool that is reclaimed for the P tiles after the K projection.
"""

import numpy as np
import ml_dtypes

import concourse.bass as bass
import concourse.tile as tile
from concourse import bacc
from concourse import mybir
from concourse.bass_utils import run_bass_kernel_spmd

B, S, HID, D = 8, 2048, 1024, 64
NCORES = 8
HCH = HID // 128          # 8 hidden chunks
KT_TILES = S // 128       # 16 k tiles
NQ = 512                  # q chunk width for the attention inner loop
QCH = S // NQ             # 4
NPAIR = KT_TILES // 2     # 8 k-tile pairs
MASK_C = 48.0             # mask offset (48 exactly representable in e4m3)

F32 = mybir.dt.float32
F16 = mybir.dt.float16
FP8 = mybir.dt.float8e4
F16_NP = np.float16
FP8_NP = ml_dtypes.float8_e4m3

LAST_EXEC_TIME_NS = None
_CACHED = {}


def _build_program(with_qk_bias=False, reps=1):
    nc = bacc.Bacc("TRN2", target_bir_lowering=False, debug=False,
                   num_swdge_queues=4)

    qT_d = nc.dram_tensor("qT", [HID, S], F16, kind="ExternalInput").ap()
    kT_d = nc.dram_tensor("kT", [HID, S], F16, kind="ExternalInput").ap()
    vT_d = nc.dram_tensor("vT", [HID, S], F16, kind="ExternalInput").ap()
    # 48*(mask.T - 1): 0 where visible, -48 where masked
    maskT_d = nc.dram_tensor("maskT", [S, S], FP8, kind="ExternalInput").ap()
    # doubled weights [W | W] so one matmul writes both partition groups
    wq_d = nc.dram_tensor("wq", [HID, 2 * D], F16, kind="ExternalInput").ap()
    wk_d = nc.dram_tensor("wk", [HID, 2 * D], F16, kind="ExternalInput").ap()
    wv_d = nc.dram_tensor("wv", [HID, 2 * D], F16, kind="ExternalInput").ap()
    if with_qk_bias:
        bq_d = nc.dram_tensor("bq", [D], F32, kind="ExternalInput").ap()
        bk_d = nc.dram_tensor("bk", [D], F32, kind="ExternalInput").ap()
    idf_d = nc.dram_tensor("idf", [128, 128], F16, kind="ExternalInput").ap()
    out_d = nc.dram_tensor("out", [S, D], F32, kind="ExternalOutput").ap()

    ExpF = mybir.ActivationFunctionType.Exp

    def _body(tc):
        with tc.tile_pool(name="const", bufs=1) as const:
            w_q = const.tile([128, HCH, 2 * D], F16, name="w_q")
            w_k = const.tile([128, HCH, 2 * D], F16, name="w_k")
            w_v = const.tile([128, HCH, 2 * D], F16, name="w_v")
            nc.sync.dma_start(w_q, wq_d.rearrange("(o p) d -> p o d", p=128))
            nc.sync.dma_start(w_k, wk_d.rearrange("(o p) d -> p o d", p=128))
            nc.sync.dma_start(w_v, wv_d.rearrange("(o p) d -> p o d", p=128))
            idf16 = const.tile([128, 128], F16, name="idf16")
            nc.sync.dma_start(idf16, idf_d)
            if with_qk_bias:
                b_q = const.tile([128, 1], F32, name="b_q")
                b_k = const.tile([128, 1], F32, name="b_k")
                nc.sync.dma_start(b_q[0:D, :], bq_d.unsqueeze(1))
                nc.sync.dma_start(b_q[64:64 + D, :], bq_d.unsqueeze(1))
                nc.sync.dma_start(b_k[0:D, :], bk_d.unsqueeze(1))
                nc.sync.dma_start(b_k[64:64 + D, :], bk_d.unsqueeze(1))
            else:
                b_q = b_k = None

            masksb = const.tile([128, KT_TILES, S], FP8, name="masksb")
            vh = const.tile([128, HCH, S], F16, name="vh")
            QT = const.tile([128, S], F16, name="QT")
            KT = const.tile([128, S], F16, name="KT")
            VT = const.tile([128, S], F16, name="VT")
            Vt = const.tile([128, KT_TILES, D + 1], F16, name="Vt")

            # ---- DMA issue plan ----
            # SP: weights, idf, q (4 x [128,8,512]); ACT queue: k then v
            # (2 x [128,8,1024] each, all issued before exp work exists);
            # gpsimd(swdge): mask (4 x [128,4,2048]).  Large descriptors
            # keep issue overhead off the compute engines.
            def dma_hid3(t, d, c0, c1, eng):
                eng.dma_start(
                    t[:, :, c0:c1],
                    d[:, c0:c1].rearrange("(o p) s -> p o s", p=128))

            ones_ap = nc.const_aps.tensor(1.0, (128, 1))

            with tc.tile_pool(name="stp", bufs=2, space="PSUM") as stp, \
                 tc.tile_pool(name="ntp", bufs=2, space="PSUM") as ntp:

                def proj(hid_t, w_t, b_t, dest, c):
                    # one 512-col chunk -> full [128, 512] PSUM (rows
                    # 64-127 duplicate 0-63 via the doubled weights)
                    cs = slice(c * NQ, (c + 1) * NQ)
                    prj = stp.tile([128, NQ], F32, name="prj", tag="prj",
                                   bufs=2)
                    for h in range(HCH):
                        nc.tensor.matmul(
                            prj, lhsT=w_t[:, h, :], rhs=hid_t[:, h, cs],
                            start=(h == 0), stop=(h == HCH - 1))
                    nc.vector.tensor_copy(dest[:, cs], prj)
                    if b_t is not None:
                        nc.vector.tensor_scalar_add(dest[:, cs], dest[:, cs],
                                                    b_t)

                # ---- staged emission (PE stream order == data arrival) --
                with tc.tile_pool(name="qkh", bufs=1) as qkh:
                    qh = qkh.tile([128, HCH, S], F16, name="qh")
                    kh = qkh.tile([128, HCH, S], F16, name="kh")

                    for c in range(QCH):
                        dma_hid3(qh, qT_d, c * NQ, (c + 1) * NQ, nc.sync)
                    for c in range(2):
                        dma_hid3(kh, kT_d, c * 1024, (c + 1) * 1024,
                                 nc.scalar)
                    for c in range(4):
                        nc.gpsimd.dma_start(
                            masksb[:, 4 * c:4 * c + 4, :],
                            maskT_d[4 * c * 128:(4 * c + 4) * 128, :]
                            .rearrange("(t p) s -> p t s", p=128))
                    for c in range(2):
                        dma_hid3(vh, vT_d, c * 1024, (c + 1) * 1024,
                                 nc.scalar)

                    for c in range(QCH):
                        proj(qh, w_q, b_q, QT, c)
                    for c in range(QCH):
                        proj(kh, w_k, b_k, KT, c)

                with tc.tile_pool(name="pre", bufs=12) as prep, \
                     tc.tile_pool(name="ptp", bufs=16) as ptp, \
                     tc.tile_pool(name="nsb", bufs=2) as nsb:

                    def sc_exp(qc, p):
                        # row-packed score pair (2p, 2p+1); DVE adds the
                        # fp8 mask offsets while casting PSUM->f16; ACT
                        # exps decoupled from the PE stream.
                        q0 = qc * NQ
                        qsl = slice(q0, q0 + NQ)
                        kta, ktb = 2 * p, 2 * p + 1
                        sa = slice(kta * 128, kta * 128 + 128)
                        sb = slice(ktb * 128, ktb * 128 + 128)
                        st = stp.tile([128, 2, NQ], F32, name="st", tag="st")
                        nc.tensor.matmul(
                            st[:, 0, :], lhsT=KT[0:D, sa], rhs=QT[0:D, qsl],
                            start=True, stop=True)
                        nc.tensor.matmul(
                            st[:, 1, :], lhsT=KT[64:64 + D, sb],
                            rhs=QT[64:64 + D, qsl], start=True, stop=True)
                        pre = prep.tile([128, 2, NQ], F16, name="pre",
                                        tag="pre")
                        nc.vector.tensor_add(pre, st,
                                             masksb[:, 2 * p:2 * p + 2, qsl])
                        pt = ptp.tile([128, 2, NQ], F16, name="pt", tag="pt")
                        nc.scalar.activation(pt, pre, ExpF)
                        return pt

                    def v_finish():
                        for kt in range(KT_TILES):
                            vtr = ntp.tile([128, D], F16, name="vtr",
                                           tag="tr")
                            nc.tensor.transpose(
                                vtr, VT[0:D, kt * 128:(kt + 1) * 128],
                                idf16[0:D, 0:D])
                            nc.vector.tensor_copy(Vt[:, kt, :D], vtr)
                            nc.vector.tensor_copy(Vt[:, kt, D:D + 1],
                                                  ones_ap)

                    def av(outT, p, pt):
                        nc.tensor.matmul(
                            outT, lhsT=Vt[:, 2 * p, :], rhs=pt[:, 0, :],
                            start=(p == 0), stop=False)
                        nc.tensor.matmul(
                            outT, lhsT=Vt[:, 2 * p + 1, :], rhs=pt[:, 1, :],
                            start=False, stop=(p == NPAIR - 1))

                    def norm(qc, outT):
                        q0 = qc * NQ
                        outT_sb = nsb.tile([D + 1, NQ], F16, name="outT_sb",
                                           tag="outT_sb")
                        nc.vector.tensor_copy(outT_sb, outT)
                        o_big = nsb.tile([128, NQ // 128, D], F32,
                                         name="o_big", tag="o_big")
                        for i in range(NQ // 128):
                            tr = ntp.tile([128, D + 1], F16, name="tr",
                                          tag="tr")
                            nc.tensor.transpose(
                                tr, outT_sb[:, i * 128:(i + 1) * 128],
                                idf16[:D + 1, :D + 1])
                            tr_sb = nsb.tile([128, D + 1], F32, name="tr_sb",
                                             tag="tr_sb")
                            nc.vector.tensor_copy(tr_sb, tr)
                            nc.vector.reciprocal(tr_sb[:, D:D + 1],
                                                 tr_sb[:, D:D + 1])
                            nc.vector.tensor_scalar_mul(
                                o_big[:, i, :], tr_sb[:, :D],
                                tr_sb[:, D:D + 1])
                        nc.sync.dma_start(
                            out_d[q0:q0 + NQ, :].rearrange(
                                "(t p) d -> p t d", p=128), o_big)

                    pts = {}
                    for qc in range(QCH):
                        for p in range(NPAIR):
                            pts[(qc, p)] = sc_exp(qc, p)
                    for c in range(QCH):
                        proj(vh, w_v, None, VT, c)
                    v_finish()
                    for qc in range(QCH):
                        outT = stp.tile([D + 1, NQ], F32, name="outT",
                                        tag="prj", bufs=2)
                        for p in range(NPAIR):
                            av(outT, p, pts[(qc, p)])
                        norm(qc, outT)

    with tile.TileContext(nc) as tc:
        if reps > 1:
            with tc.For_i(0, reps, 1):
                _body(tc)
        else:
            _body(tc)

    nc.compile()
    return nc


def _prep_inputs(q_hidden_inputs, k_hidden_inputs, v_hidden_inputs, mask,
                 Wq, bq, Wk, bk, Wv, bv):
    scale = np.float32(1.0 / np.sqrt(np.float32(D)))
    wq = (np.asarray(Wq, np.float32) * scale).astype(F16_NP)
    wk = np.asarray(Wk, np.float32).astype(F16_NP)
    wv = np.asarray(Wv, np.float32).astype(F16_NP)
    wq2 = np.ascontiguousarray(np.concatenate([wq, wq], axis=1))
    wk2 = np.ascontiguousarray(np.concatenate([wk, wk], axis=1))
    wv2 = np.ascontiguousarray(np.concatenate([wv, wv], axis=1))
    bqs = (np.asarray(bq, np.float32) * scale)
    bks = np.asarray(bk, np.float32)
    with_qk_bias = bool(np.any(bqs != 0) or np.any(bks != 0))
    idf = np.eye(128, dtype=np.float32).astype(F16_NP)

    q = np.asarray(q_hidden_inputs, np.float32)
    k = np.asarray(k_hidden_inputs, np.float32)
    v = np.asarray(v_hidden_inputs, np.float32)
    m = np.asarray(mask)

    in_maps = []
    for b in range(B):
        im = {
            "qT": np.ascontiguousarray(q[b].T).astype(F16_NP),
            "kT": np.ascontiguousarray(k[b].T).astype(F16_NP),
            "vT": np.ascontiguousarray(v[b].T).astype(F16_NP),
            "maskT": ((np.ascontiguousarray(m[b].T) - np.int32(1)) *
                      np.float32(MASK_C)).astype(FP8_NP),
            "wq": wq2, "wk": wk2, "wv": wv2,
            "idf": idf,
        }
        if with_qk_bias:
            im["bq"] = bqs
            im["bk"] = bks
        in_maps.append(im)
    return in_maps, with_qk_bias


def kernel(q_hidden_inputs, k_hidden_inputs, v_hidden_inputs, mask,
           Wq, bq, Wk, bk, Wv, bv, trace=False):
    global LAST_EXEC_TIME_NS
    in_maps, with_qk_bias = _prep_inputs(
        q_hidden_inputs, k_hidden_inputs, v_hidden_inputs,
        mask, Wq, bq, Wk, bk, Wv, bv)
    key = ("nc", with_qk_bias)
    if key not in _CACHED:
        _CACHED[key] = _build_program(with_qk_bias)
    nc = _CACHED[key]

    res = run_bass_kernel_spmd(nc, in_maps, list(range(NCORES)), trace=trace)
    LAST_EXEC_TIME_NS = res.exec_time_ns
    out = np.stack([res.results[b]["out"] for b in range(B)], axis=0)
    # bv folds into the output exactly: softmax rows sum to 1, so
    # attn @ (V + 1 bv^T) = attn @ V + bv.
    out = out + np.asarray(bv, np.float32)[None, None, :]
    return out


# revision 11
# speedup vs baseline: 1.0428x; 1.0241x over previous
"""Trainium2 Bass kernel for masked single-head attention.

Reference computation (per batch b):
    Q = q_hidden[b] @ Wq + bq            # [S, D]
    K = k_hidden[b] @ Wk + bk            # [S, D]
    V = v_hidden[b] @ Wv + bv            # [S, D]
    S_qk = (Q @ K.T) / sqrt(D)           # [S, S]
    S_qk = where(mask[b]==0, -1e9, S_qk)
    out[b] = softmax(S_qk, -1) @ V       # [S, D]

Sharding: data-parallel over batch, one batch per NeuronCore (B == 8 cores).
No collectives.

Device-side dataflow (per core, S=2048, HID=1024, D=64).  The kernel is
purely TensorE-row-stream bound, so the design minimizes total matmul
moving-rows (cost per matmul = out free size, regardless of contraction
or partition count):
  - host ships transposed hiddens qT/kT/vT [HID, S] fp16, the mask as
    48*(mT-1) in fp8 {0,-48}, and doubled weights [W|W] [HID, 2D] so one
    projection pass writes both PSUM partition groups 0-63 / 64-127
    (the row-duplicated layout the row-packed score matmuls need) in a
    single N-row stream.  Wq is pre-scaled by 1/sqrt(D).
  - projections on PE: per 512-col chunk, 8 contraction chunks,
    lhsT=[W|W][128,128] -> PSUM [128,512]; one DVE cast to f16
    QT/KT/VT [128, S] (rows 64-127 duplicate 0-63).
  - scores^T for k-tile pair (2j, 2j+1) land in one [128, 1024] PSUM
    tile via two row-packed f16 matmuls (contraction D=64 on partitions
    0-63 / 64-127).
  - mask is applied by DVE: pre = st + 48*(mT-1)  (PSUM f32 + fp8 ->
    f16 SBUF).  This frees the PSUM tile at DVE speed and costs the PE
    nothing.  exp runs on ScalarE (ACT) decoupled from the PE stream:
    masked entries become exp(s-48) which underflows f16 to exact 0.
  - out^T[65, q] += [V|1].T @ P^T accumulated over k-tiles: rows 0..63
    numerator, row 64 the softmax denominator.
  - norm: cast outT to f16, PE-transpose [65,128] slices back to
    [128,65], reciprocal of the [128,1] denominator column, multiply,
    DMA out [q, 64] as f32.
  - DMA: few large descriptors; q on SP (first), k/v on ACT queue
    (early, before exp starts), mask on gpsimd.  qh/kh live in a scoped
    pool that is reclaimed for the P tiles after the K projection.
"""

import numpy as np
import ml_dtypes

import concourse.bass as bass
import concourse.tile as tile
from concourse import bacc
from concourse import mybir
from concourse.bass_utils import run_bass_kernel_spmd

B, S, HID, D = 8, 2048, 1024, 64
NCORES = 8
HCH = HID // 128          # 8 hidden chunks
KT_TILES = S // 128       # 16 k tiles
NQ = 512                  # q chunk width for the attention inner loop
QCH = S // NQ             # 4
NPAIR = KT_TILES // 2     # 8 k-tile pairs
MASK_C = 48.0             # mask offset (48 exactly representable in e4m3)

F32 = mybir.dt.float32
F16 = mybir.dt.float16
FP8 = mybir.dt.float8e4
F16_NP = np.float16
FP8_NP = ml_dtypes.float8_e4m3

LAST_EXEC_TIME_NS = None
_CACHED = {}


def _build_program(with_qk_bias=False, reps=1):
    nc = bacc.Bacc("TRN2", target_bir_lowering=False, debug=False,
                   num_swdge_queues=4)

    qT_d = nc.dram_tensor("qT", [HID, S], F16, kind="ExternalInput").ap()
    kT_d = nc.dram_tensor("kT", [HID, S], F16, kind="ExternalInput").ap()
    vT_d = nc.dram_tensor("vT", [HID, S], F16, kind="ExternalInput").ap()
    # 48*(mask.T - 1): 0 where visible, -48 where masked
    maskT_d = nc.dram_tensor("maskT", [S, S], FP8, kind="ExternalInput").ap()
    # all three doubled weights [W | W], packed partition-major so one 2D
    # DMA ships them: wall[p, (j*8+h)*128 + d] = W2_j[h*128+p, d]
    wall_d = nc.dram_tensor("wall", [128, 3 * HCH * 128], F16,
                            kind="ExternalInput").ap()
    idm_d = nc.dram_tensor("idm", [128, 128], FP8, kind="ExternalInput").ap()
    if with_qk_bias:
        bq_d = nc.dram_tensor("bq", [D], F32, kind="ExternalInput").ap()
        bk_d = nc.dram_tensor("bk", [D], F32, kind="ExternalInput").ap()
    idf_d = nc.dram_tensor("idf", [128, 128], F16, kind="ExternalInput").ap()
    out_d = nc.dram_tensor("out", [S, D], F32, kind="ExternalOutput").ap()

    ExpF = mybir.ActivationFunctionType.Exp

    def _body(tc):
        with tc.tile_pool(name="const", bufs=1) as const:
            w_all = const.tile([128, 3, HCH, 2 * D], F16, name="w_all")
            nc.sync.dma_start(w_all, wall_d.rearrange(
                "p (j o d) -> p j o d", j=3, o=HCH))
            w_q, w_k, w_v = (w_all[:, j] for j in range(3))
            idf16 = const.tile([128, 128], F16, name="idf16")
            idm = const.tile([128, 128], FP8, name="idm")
            if with_qk_bias:
                b_q = const.tile([128, 1], F32, name="b_q")
                b_k = const.tile([128, 1], F32, name="b_k")
                nc.sync.dma_start(b_q[0:D, :], bq_d.unsqueeze(1))
                nc.sync.dma_start(b_q[64:64 + D, :], bq_d.unsqueeze(1))
                nc.sync.dma_start(b_k[0:D, :], bk_d.unsqueeze(1))
                nc.sync.dma_start(b_k[64:64 + D, :], bk_d.unsqueeze(1))
            else:
                b_q = b_k = None

            masksb = const.tile([128, KT_TILES, S], FP8, name="masksb")
            vh = const.tile([128, HCH, S], F16, name="vh")
            QT = const.tile([128, S], F16, name="QT")
            KT = const.tile([128, S], F16, name="KT")
            VT = const.tile([128, S], F16, name="VT")
            Vt = const.tile([128, KT_TILES, D + 1], F16, name="Vt")

            # 2D DMA descriptors only (3D falls back to slow swdge):
            # per hidden chunk h, a [128, span] slice.
            def dma_hid(t, d, c0, c1, eng):
                csl = slice(c0, c1)
                for h in range(HCH):
                    eng.dma_start(t[:, h, csl],
                                  d[h * 128:(h + 1) * 128, csl])

            ones_ap = nc.const_aps.tensor(1.0, (128, 1))

            with tc.tile_pool(name="stp", bufs=2, space="PSUM") as stp, \
                 tc.tile_pool(name="ntp", bufs=2, space="PSUM") as ntp:

                def proj(hid_t, w_t, b_t, dest, c):
                    # one 512-col chunk -> full [128, 512] PSUM (rows
                    # 64-127 duplicate 0-63 via the doubled weights)
                    cs = slice(c * NQ, (c + 1) * NQ)
                    prj = stp.tile([128, NQ], F32, name="prj", tag="prj",
                                   bufs=2)
                    for h in range(HCH):
                        nc.tensor.matmul(
                            prj, lhsT=w_t[:, h, :], rhs=hid_t[:, h, cs],
                            start=(h == 0), stop=(h == HCH - 1))
                    nc.vector.tensor_copy(dest[:, cs], prj)
                    if b_t is not None:
                        nc.vector.tensor_scalar_add(dest[:, cs], dest[:, cs],
                                                    b_t)

                # ---- staged emission (PE stream order == data arrival) --
                with tc.tile_pool(name="qkh", bufs=1) as qkh:
                    qh = qkh.tile([128, HCH, S], F16, name="qh")
                    kh = qkh.tile([128, HCH, S], F16, name="kh")

                    # SP: w, q; ACT queue: k then v (issued before any exp
                    # exists on that engine); gpsimd(swdge): mask tiles.
                    for c in range(2):
                        dma_hid(qh, qT_d, c * 1024, (c + 1) * 1024, nc.sync)
                    for c in range(2):
                        dma_hid(kh, kT_d, c * 1024, (c + 1) * 1024,
                                nc.scalar)
                    for kt in range(KT_TILES):
                        nc.gpsimd.dma_start(
                            masksb[:, kt, :],
                            maskT_d[kt * 128:(kt + 1) * 128, :])
                    nc.sync.dma_start(idm, idm_d)
                    nc.sync.dma_start(idf16, idf_d)
                    for c in range(2):
                        dma_hid(vh, vT_d, c * 1024, (c + 1) * 1024,
                                nc.scalar)

                    for c in range(QCH):
                        proj(qh, w_q, b_q, QT, c)
                    for c in range(QCH):
                        proj(kh, w_k, b_k, KT, c)

                with tc.tile_pool(name="pre", bufs=16) as prep, \
                     tc.tile_pool(name="ptp", bufs=28) as ptp, \
                     tc.tile_pool(name="nsb", bufs=2) as nsb:

                    def sc_unit(qc, p, even):
                        # row-packed score pair (2p, 2p+1).  Even units
                        # accumulate the fp8 mask offsets on the PE
                        # (idm @ moffs) and drain via direct ACT exp;
                        # odd units drain via a DVE add of the mask
                        # offsets, with the exp deferred.  This splits
                        # the PSUM-drain load across both engines.
                        q0 = qc * NQ
                        qsl = slice(q0, q0 + NQ)
                        kta, ktb = 2 * p, 2 * p + 1
                        sa = slice(kta * 128, kta * 128 + 128)
                        sb = slice(ktb * 128, ktb * 128 + 128)
                        st = stp.tile([128, 2, NQ], F32, name="st", tag="st")
                        nc.tensor.matmul(
                            st[:, 0, :], lhsT=KT[0:D, sa], rhs=QT[0:D, qsl],
                            start=True, stop=not even)
                        if even:
                            nc.tensor.matmul(
                                st[:, 0, :], lhsT=idm,
                                rhs=masksb[:, kta, qsl],
                                start=False, stop=True)
                        nc.tensor.matmul(
                            st[:, 1, :], lhsT=KT[64:64 + D, sb],
                            rhs=QT[64:64 + D, qsl], start=True,
                            stop=not even)
                        if even:
                            nc.tensor.matmul(
                                st[:, 1, :], lhsT=idm,
                                rhs=masksb[:, ktb, qsl],
                                start=False, stop=True)
                            pt = ptp.tile([128, 2, NQ], F16, name="pt",
                                          tag="pt")
                            nc.scalar.activation(pt, st, ExpF)
                            return pt
                        pre = prep.tile([128, 2, NQ], F16, name="pre",
                                        tag="pre")
                        nc.vector.tensor_add(pre, st,
                                             masksb[:, 2 * p:2 * p + 2, qsl])
                        return pre

                    def v_fin(kt):
                        vtr = ntp.tile([128, D], F16, name="vtr", tag="tr")
                        nc.tensor.transpose(
                            vtr, VT[0:D, kt * 128:(kt + 1) * 128],
                            idf16[0:D, 0:D])
                        nc.vector.tensor_copy(Vt[:, kt, :D], vtr)
                        nc.vector.tensor_copy(Vt[:, kt, D:D + 1], ones_ap)

                    def av(outT, p, pt):
                        nc.tensor.matmul(
                            outT, lhsT=Vt[:, 2 * p, :], rhs=pt[:, 0, :],
                            start=(p == 0), stop=False)
                        nc.tensor.matmul(
                            outT, lhsT=Vt[:, 2 * p + 1, :], rhs=pt[:, 1, :],
                            start=False, stop=(p == NPAIR - 1))

                    def norm(qc, outT):
                        q0 = qc * NQ
                        outT_sb = nsb.tile([D + 1, NQ], F16, name="outT_sb",
                                           tag="outT_sb")
                        nc.vector.tensor_copy(outT_sb, outT)
                        o_big = nsb.tile([128, NQ // 128, D], F32,
                                         name="o_big", tag="o_big")
                        for i in range(NQ // 128):
                            tr = ntp.tile([128, D + 1], F16, name="tr",
                                          tag="tr")
                            nc.tensor.transpose(
                                tr, outT_sb[:, i * 128:(i + 1) * 128],
                                idf16[:D + 1, :D + 1])
                            rcp = nsb.tile([128, 1], F32, name="rcp",
                                           tag="rcp")
                            nc.vector.reciprocal(rcp, tr[:, D:D + 1])
                            nc.vector.tensor_scalar_mul(
                                o_big[:, i, :], tr[:, :D], rcp)
                        nc.sync.dma_start(
                            out_d[q0:q0 + NQ, :].rearrange(
                                "(t p) d -> p t d", p=128), o_big)

                    pts = {}
                    pres = {}
                    for qc in range(QCH):
                        for p in range(NPAIR):
                            even = (qc * NPAIR + p) % 2 == 0
                            r = sc_unit(qc, p, even)
                            if even:
                                pts[(qc, p)] = r
                            else:
                                pres[(qc, p)] = r
                    for c in range(QCH):
                        proj(vh, w_v, None, VT, c)
                        for kt in range(4 * c, 4 * c + 4):
                            v_fin(kt)
                    # deferred exps for the odd units (ACT, off the PE
                    # critical path)
                    for qc in range(QCH):
                        for p in range(NPAIR):
                            if (qc, p) in pres:
                                pt = ptp.tile([128, 2, NQ], F16, name="pt",
                                              tag="pt")
                                nc.scalar.activation(pt, pres[(qc, p)], ExpF)
                                pts[(qc, p)] = pt
                    for qc in range(QCH):
                        outT = stp.tile([D + 1, NQ], F32, name="outT",
                                        tag="prj", bufs=2)
                        for p in range(NPAIR):
                            av(outT, p, pts[(qc, p)])
                        norm(qc, outT)

    with tile.TileContext(nc) as tc:
        if reps > 1:
            with tc.For_i(0, reps, 1):
                _body(tc)
        else:
            _body(tc)

    nc.compile()
    return nc


def _prep_inputs(q_hidden_inputs, k_hidden_inputs, v_hidden_inputs, mask,
                 Wq, bq, Wk, bk, Wv, bv):
    scale = np.float32(1.0 / np.sqrt(np.float32(D)))
    wq = (np.asarray(Wq, np.float32) * scale).astype(F16_NP)
    wk = np.asarray(Wk, np.float32).astype(F16_NP)
    wv = np.asarray(Wv, np.float32).astype(F16_NP)
    wq2 = np.concatenate([wq, wq], axis=1)
    wk2 = np.concatenate([wk, wk], axis=1)
    wv2 = np.concatenate([wv, wv], axis=1)
    # wall[p, (j*8+h)*128 + d] = W2_j[h*128+p, d]
    wall = np.ascontiguousarray(
        np.stack([wq2, wk2, wv2])               # [3, HID, 2D]
        .reshape(3, HCH, 128, 2 * D)            # [3, h, p, d]
        .transpose(2, 0, 1, 3)                  # [p, 3, h, d]
        .reshape(128, 3 * HCH * 2 * D))
    bqs = (np.asarray(bq, np.float32) * scale)
    bks = np.asarray(bk, np.float32)
    with_qk_bias = bool(np.any(bqs != 0) or np.any(bks != 0))
    idf = np.eye(128, dtype=np.float32).astype(F16_NP)
    idm = np.eye(128, dtype=np.float32).astype(FP8_NP)

    q = np.asarray(q_hidden_inputs, np.float32)
    k = np.asarray(k_hidden_inputs, np.float32)
    v = np.asarray(v_hidden_inputs, np.float32)
    m = np.asarray(mask)

    in_maps = []
    for b in range(B):
        im = {
            "qT": np.ascontiguousarray(q[b].T).astype(F16_NP),
            "kT": np.ascontiguousarray(k[b].T).astype(F16_NP),
            "vT": np.ascontiguousarray(v[b].T).astype(F16_NP),
            "maskT": ((np.ascontiguousarray(m[b].T) - np.int32(1)) *
                      np.float32(MASK_C)).astype(FP8_NP),
            "wall": wall, "idm": idm,
            "idf": idf,
        }
        if with_qk_bias:
            im["bq"] = bqs
            im["bk"] = bks
        in_maps.append(im)
    return in_maps, with_qk_bias


def kernel(q_hidden_inputs, k_hidden_inputs, v_hidden_inputs, mask,
           Wq, bq, Wk, bk, Wv, bv, trace=False):
    global LAST_EXEC_TIME_NS
    in_maps, with_qk_bias = _prep_inputs(
        q_hidden_inputs, k_hidden_inputs, v_hidden_inputs,
        mask, Wq, bq, Wk, bk, Wv, bv)
    key = ("nc", with_qk_bias)
    if key not in _CACHED:
        _CACHED[key] = _build_program(with_qk_bias)
    nc = _CACHED[key]

    res = run_bass_kernel_spmd(nc, in_maps, list(range(NCORES)), trace=trace)
    LAST_EXEC_TIME_NS = res.exec_time_ns
    out = np.stack([res.results[b]["out"] for b in range(B)], axis=0)
    # bv folds into the output exactly: softmax rows sum to 1, so
    # attn @ (V + 1 bv^T) = attn @ V + bv.
    out = out + np.asarray(bv, np.float32)[None, None, :]
    return out


# revision 15
# speedup vs baseline: 1.0825x; 1.0381x over previous
"""Trainium2 Bass kernel for masked single-head attention.

Reference computation (per batch b):
    Q = q_hidden[b] @ Wq + bq            # [S, D]
    K = k_hidden[b] @ Wk + bk            # [S, D]
    V = v_hidden[b] @ Wv + bv            # [S, D]
    S_qk = (Q @ K.T) / sqrt(D)           # [S, S]
    S_qk = where(mask[b]==0, -1e9, S_qk)
    out[b] = softmax(S_qk, -1) @ V       # [S, D]

Sharding: data-parallel over batch, one batch per NeuronCore (B == 8 cores).
No collectives.

Device-side dataflow (per core, S=2048, HID=1024, D=64).  The kernel is
purely TensorE-row-stream bound, so the design minimizes total matmul
moving-rows (cost per matmul = out free size, regardless of contraction
or partition count):
  - host ships transposed hiddens qT/kT/vT [HID, S] fp16, the mask as
    48*(mT-1) in fp8 {0,-48}, and doubled weights [W|W] [HID, 2D] so one
    projection pass writes both PSUM partition groups 0-63 / 64-127
    (the row-duplicated layout the row-packed score matmuls need) in a
    single N-row stream.  Wq is pre-scaled by 1/sqrt(D).
  - projections on PE: per 512-col chunk, 8 contraction chunks,
    lhsT=[W|W][128,128] -> PSUM [128,512]; one DVE cast to f16
    QT/KT/VT [128, S] (rows 64-127 duplicate 0-63).
  - scores^T for k-tile pair (2j, 2j+1) land in one [128, 1024] PSUM
    tile via two row-packed f16 matmuls (contraction D=64 on partitions
    0-63 / 64-127).
  - mask is applied by DVE: pre = st + 48*(mT-1)  (PSUM f32 + fp8 ->
    f16 SBUF).  This frees the PSUM tile at DVE speed and costs the PE
    nothing.  exp runs on ScalarE (ACT) decoupled from the PE stream:
    masked entries become exp(s-48) which underflows f16 to exact 0.
  - out^T[65, q] += [V|1].T @ P^T accumulated over k-tiles: rows 0..63
    numerator, row 64 the softmax denominator.
  - norm: cast outT to f16, PE-transpose [65,128] slices back to
    [128,65], reciprocal of the [128,1] denominator column, multiply,
    DMA out [q, 64] as f32.
  - DMA: few large descriptors; q on SP (first), k/v on ACT queue
    (early, before exp starts), mask on gpsimd.  qh/kh live in a scoped
    pool that is reclaimed for the P tiles after the K projection.
"""

import numpy as np
import ml_dtypes

import concourse.bass as bass
import concourse.tile as tile
from concourse import bacc
from concourse import mybir
from concourse.bass_utils import run_bass_kernel_spmd

B, S, HID, D = 8, 2048, 1024, 64
NCORES = 8
HCH = HID // 128          # 8 hidden chunks
KT_TILES = S // 128       # 16 k tiles
NQ = 512                  # q chunk width for the attention inner loop
QCH = S // NQ             # 4
NPAIR = KT_TILES // 2     # 8 k-tile pairs
MASK_C = 48.0             # mask offset (48 exactly representable in e4m3)

F32 = mybir.dt.float32
F16 = mybir.dt.float16
FP8 = mybir.dt.float8e4
F16_NP = np.float16
FP8_NP = ml_dtypes.float8_e4m3

LAST_EXEC_TIME_NS = None
_CACHED = {}


def _build_program(with_qk_bias=False, reps=1):
    nc = bacc.Bacc("TRN2", target_bir_lowering=False, debug=False,
                   num_swdge_queues=4)

    qT_d = nc.dram_tensor("qT", [HID, S], F16, kind="ExternalInput").ap()
    kT_d = nc.dram_tensor("kT", [HID, S], F16, kind="ExternalInput").ap()
    vT_d = nc.dram_tensor("vT", [HID, S], F16, kind="ExternalInput").ap()
    # 48*(mask.T - 1): 0 where visible, -48 where masked
    maskT_d = nc.dram_tensor("maskT", [S, S], FP8, kind="ExternalInput").ap()
    # all three doubled weights [W | W], packed partition-major so one 2D
    # DMA ships them: wall[p, (j*8+h)*128 + d] = W2_j[h*128+p, d]
    wall_d = nc.dram_tensor("wall", [128, 3 * HCH * 128], F16,
                            kind="ExternalInput").ap()
    idm_d = nc.dram_tensor("idm", [128, 128], FP8, kind="ExternalInput").ap()
    if with_qk_bias:
        bq_d = nc.dram_tensor("bq", [D], F32, kind="ExternalInput").ap()
        bk_d = nc.dram_tensor("bk", [D], F32, kind="ExternalInput").ap()
    idf_d = nc.dram_tensor("idf", [128, 128], F16, kind="ExternalInput").ap()
    # [qc, p, t*D+d] layout so the output DMA is a clean 2D descriptor;
    # host untangles with a reshape/transpose.
    out_d = nc.dram_tensor("out", [QCH, 128, (NQ // 128) * D], F32,
                           kind="ExternalOutput").ap()

    ExpF = mybir.ActivationFunctionType.Exp

    def _body(tc):
        with tc.tile_pool(name="const", bufs=1) as const:
            w_all = const.tile([128, 3, HCH, 2 * D], F16, name="w_all")
            nc.sync.dma_start(w_all, wall_d.rearrange(
                "p (j o d) -> p j o d", j=3, o=HCH))
            w_q, w_k, w_v = (w_all[:, j] for j in range(3))
            idf16 = const.tile([128, 128], F16, name="idf16")
            idm = const.tile([128, 128], FP8, name="idm")
            if with_qk_bias:
                b_q = const.tile([128, 1], F32, name="b_q")
                b_k = const.tile([128, 1], F32, name="b_k")
                nc.sync.dma_start(b_q[0:D, :], bq_d.unsqueeze(1))
                nc.sync.dma_start(b_q[64:64 + D, :], bq_d.unsqueeze(1))
                nc.sync.dma_start(b_k[0:D, :], bk_d.unsqueeze(1))
                nc.sync.dma_start(b_k[64:64 + D, :], bk_d.unsqueeze(1))
            else:
                b_q = b_k = None

            masksb = const.tile([128, KT_TILES, S], FP8, name="masksb")
            vh = const.tile([128, HCH, S], F16, name="vh")
            QT = const.tile([128, S], F16, name="QT")
            KT = const.tile([128, S], F16, name="KT")
            VT = const.tile([128, S], F16, name="VT")
            Vt = const.tile([128, KT_TILES, D + 1], F16, name="Vt")

            # full-row 2D DMA descriptors: [128, S] per hidden chunk gives
            # 4KB contiguous runs per partition (2KB descriptors halve the
            # per-queue bandwidth).
            def dma_hid(t, d, eng):
                for h in range(HCH):
                    eng.dma_start(t[:, h, :], d[h * 128:(h + 1) * 128, :])

            with tc.tile_pool(name="stp", bufs=2, space="PSUM") as stp:

                def proj(hid_t, w_t, b_t, dest, c, copy_eng):
                    # one 512-col chunk -> full [128, 512] PSUM (rows
                    # 64-127 duplicate 0-63 via the doubled weights)
                    cs = slice(c * NQ, (c + 1) * NQ)
                    prj = stp.tile([128, NQ], F32, name="prj", tag="prj",
                                   bufs=2)
                    for h in range(HCH):
                        nc.tensor.matmul(
                            prj, lhsT=w_t[:, h, :], rhs=hid_t[:, h, cs],
                            start=(h == 0), stop=(h == HCH - 1))
                    copy_eng.tensor_copy(dest[:, cs], prj)
                    if b_t is not None:
                        copy_eng.tensor_scalar_add(dest[:, cs], dest[:, cs],
                                                   b_t)

                # ---- staged emission (PE stream order == data arrival) --
                with tc.tile_pool(name="qkh", bufs=1) as qkh:
                    qh = qkh.tile([128, HCH, S], F16, name="qh")
                    kh = qkh.tile([128, HCH, S], F16, name="kh")

                    # SP queue: w, q; ACT queue: k then v (issued before
                    # any exp exists there); gpsimd(swdge): mask tiles.
                    nc.sync.dma_start(idm, idm_d)
                    nc.sync.dma_start(idf16, idf_d)
                    dma_hid(qh, qT_d, nc.sync)
                    dma_hid(kh, kT_d, nc.scalar)
                    for kt in range(KT_TILES):
                        nc.gpsimd.dma_start(
                            masksb[:, kt, :],
                            maskT_d[kt * 128:(kt + 1) * 128, :])
                    dma_hid(vh, vT_d, nc.scalar)

                    for c in range(QCH):
                        proj(qh, w_q, b_q, QT, c, nc.vector)
                    for c in range(QCH):
                        proj(kh, w_k, b_k, KT, c, nc.vector)

                with tc.tile_pool(name="pre", bufs=32) as prep, \
                     tc.tile_pool(name="ptp", bufs=56) as ptp, \
                     tc.tile_pool(name="nsb", bufs=2) as nsb:

                    def sc_unit(qc, p):
                        # row-packed score pair (2p, 2p+1), one PSUM bank
                        # per k-tile so the drain pipeline runs 5 deep.
                        # Half a: mask offsets accumulated on the PE (fp8
                        # idm @ moffs), drained by a direct ACT exp.
                        # Half b: drained by a DVE add of the mask
                        # offsets; its exp is deferred.  Drain load is
                        # split evenly across ACT and DVE every unit.
                        q0 = qc * NQ
                        qsl = slice(q0, q0 + NQ)
                        kta, ktb = 2 * p, 2 * p + 1
                        sa = slice(kta * 128, kta * 128 + 128)
                        sb = slice(ktb * 128, ktb * 128 + 128)
                        sta = stp.tile([128, NQ], F32, name="sta", tag="st",
                                       bufs=5)
                        nc.tensor.matmul(
                            sta, lhsT=KT[0:D, sa], rhs=QT[0:D, qsl],
                            start=True, stop=False)
                        nc.tensor.matmul(
                            sta, lhsT=idm, rhs=masksb[:, kta, qsl],
                            start=False, stop=True)
                        pta = ptp.tile([128, NQ], F16, name="pta", tag="pt")
                        nc.scalar.activation(pta, sta, ExpF)
                        stb = stp.tile([128, NQ], F32, name="stb", tag="st",
                                       bufs=5)
                        nc.tensor.matmul(
                            stb, lhsT=KT[64:64 + D, sb],
                            rhs=QT[64:64 + D, qsl], start=True, stop=True)
                        pre = prep.tile([128, NQ], F16, name="pre",
                                        tag="pre")
                        nc.vector.tensor_add(pre, stb, masksb[:, ktb, qsl])
                        return pta, pre

                    def v_fin(kt):
                        vtr = stp.tile([128, D], F16, name="vtr", tag="prj",
                                       bufs=2)
                        nc.tensor.transpose(
                            vtr, VT[0:D, kt * 128:(kt + 1) * 128],
                            idf16[0:D, 0:D])
                        nc.vector.tensor_copy(Vt[:, kt, :D], vtr)

                    def av(outT, p, pta, ptb):
                        nc.tensor.matmul(
                            outT, lhsT=Vt[:, 2 * p, :], rhs=pta,
                            start=(p == 0), stop=False)
                        nc.tensor.matmul(
                            outT, lhsT=Vt[:, 2 * p + 1, :], rhs=ptb,
                            start=False, stop=(p == NPAIR - 1))

                    def norm(qc, outT):
                        outT_sb = nsb.tile([D + 1, NQ], F16, name="outT_sb",
                                           tag="outT_sb")
                        nc.vector.tensor_copy(outT_sb, outT)
                        o_big = nsb.tile([128, (NQ // 128) * D], F32,
                                         name="o_big", tag="o_big")
                        for i in range(NQ // 128):
                            tr = stp.tile([128, D + 1], F16, name="tr",
                                          tag="prj", bufs=2)
                            nc.tensor.transpose(
                                tr, outT_sb[:, i * 128:(i + 1) * 128],
                                idf16[:D + 1, :D + 1])
                            rcp = nsb.tile([128, 1], F32, name="rcp",
                                           tag="rcp")
                            nc.vector.reciprocal(rcp, tr[:, D:D + 1])
                            nc.vector.tensor_scalar_mul(
                                o_big[:, i * D:(i + 1) * D], tr[:, :D], rcp)
                        nc.sync.dma_start(out_d[qc], o_big)

                    # ones column of Vt written once
                    nc.gpsimd.memset(Vt[:, :, D:D + 1], 1.0)

                    pts = {}
                    pres = {}
                    for qc in range(QCH):
                        for p in range(NPAIR):
                            pts[(qc, p)], pres[(qc, p)] = sc_unit(qc, p)
                    for c in range(QCH):
                        proj(vh, w_v, None, VT, c, nc.vector)
                        for kt in range(4 * c, 4 * c + 4):
                            v_fin(kt)
                    # deferred exps for the b-halves (ACT, off the PE
                    # critical path)
                    ptbs = {}
                    for qc in range(QCH):
                        for p in range(NPAIR):
                            ptb = ptp.tile([128, NQ], F16, name="ptb",
                                           tag="pt")
                            nc.scalar.activation(ptb, pres[(qc, p)], ExpF)
                            ptbs[(qc, p)] = ptb
                    for qc in range(QCH):
                        outT = stp.tile([D + 1, NQ], F32, name="outT",
                                        tag="outT", bufs=1)
                        for p in range(NPAIR):
                            av(outT, p, pts[(qc, p)], ptbs[(qc, p)])
                        norm(qc, outT)

    with tile.TileContext(nc) as tc:
        if reps > 1:
            with tc.For_i(0, reps, 1):
                _body(tc)
        else:
            _body(tc)

    nc.compile()
    return nc


def _prep_inputs(q_hidden_inputs, k_hidden_inputs, v_hidden_inputs, mask,
                 Wq, bq, Wk, bk, Wv, bv):
    scale = np.float32(1.0 / np.sqrt(np.float32(D)))
    wq = (np.asarray(Wq, np.float32) * scale).astype(F16_NP)
    wk = np.asarray(Wk, np.float32).astype(F16_NP)
    wv = np.asarray(Wv, np.float32).astype(F16_NP)
    wq2 = np.concatenate([wq, wq], axis=1)
    wk2 = np.concatenate([wk, wk], axis=1)
    wv2 = np.concatenate([wv, wv], axis=1)
    # wall[p, (j*8+h)*128 + d] = W2_j[h*128+p, d]
    wall = np.ascontiguousarray(
        np.stack([wq2, wk2, wv2])               # [3, HID, 2D]
        .reshape(3, HCH, 128, 2 * D)            # [3, h, p, d]
        .transpose(2, 0, 1, 3)                  # [p, 3, h, d]
        .reshape(128, 3 * HCH * 2 * D))
    bqs = (np.asarray(bq, np.float32) * scale)
    bks = np.asarray(bk, np.float32)
    with_qk_bias = bool(np.any(bqs != 0) or np.any(bks != 0))
    idf = np.eye(128, dtype=np.float32).astype(F16_NP)
    idm = np.eye(128, dtype=np.float32).astype(FP8_NP)

    q = np.asarray(q_hidden_inputs, np.float32)
    k = np.asarray(k_hidden_inputs, np.float32)
    v = np.asarray(v_hidden_inputs, np.float32)
    m = np.asarray(mask)

    in_maps = []
    for b in range(B):
        im = {
            "qT": np.ascontiguousarray(q[b].T).astype(F16_NP),
            "kT": np.ascontiguousarray(k[b].T).astype(F16_NP),
            "vT": np.ascontiguousarray(v[b].T).astype(F16_NP),
            "maskT": ((np.ascontiguousarray(m[b].T) - np.int32(1)) *
                      np.float32(MASK_C)).astype(FP8_NP),
            "wall": wall, "idm": idm,
            "idf": idf,
        }
        if with_qk_bias:
            im["bq"] = bqs
            im["bk"] = bks
        in_maps.append(im)
    return in_maps, with_qk_bias


def kernel(q_hidden_inputs, k_hidden_inputs, v_hidden_inputs, mask,
           Wq, bq, Wk, bk, Wv, bv, trace=False):
    global LAST_EXEC_TIME_NS
    in_maps, with_qk_bias = _prep_inputs(
        q_hidden_inputs, k_hidden_inputs, v_hidden_inputs,
        mask, Wq, bq, Wk, bk, Wv, bv)
    key = ("nc", with_qk_bias)
    if key not in _CACHED:
        _CACHED[key] = _build_program(with_qk_bias)
    nc = _CACHED[key]

    res = run_bass_kernel_spmd(nc, in_maps, list(range(NCORES)), trace=trace)
    LAST_EXEC_TIME_NS = res.exec_time_ns
    # out_d is [qc, p, t*D+d] with q = qc*512 + t*128 + p
    out = np.stack(
        [res.results[b]["out"].reshape(QCH, 128, NQ // 128, D)
         .transpose(0, 2, 1, 3).reshape(S, D) for b in range(B)], axis=0)
    # bv folds into the output exactly: softmax rows sum to 1, so
    # attn @ (V + 1 bv^T) = attn @ V + bv.
    out = out + np.asarray(bv, np.float32)[None, None, :]
    return out


# revision 20
# speedup vs baseline: 1.1577x; 1.0694x over previous
"""Trainium2 Bass kernel for masked single-head attention.

Reference computation (per batch b):
    Q = q_hidden[b] @ Wq + bq            # [S, D]
    K = k_hidden[b] @ Wk + bk            # [S, D]
    V = v_hidden[b] @ Wv + bv            # [S, D]
    S_qk = (Q @ K.T) / sqrt(D)           # [S, S]
    S_qk = where(mask[b]==0, -1e9, S_qk)
    out[b] = softmax(S_qk, -1) @ V       # [S, D]

Sharding: data-parallel over batch, one batch per NeuronCore (B == 8 cores).
No collectives.

Device-side dataflow (per core, S=2048, HID=1024, D=64).  The kernel is
purely TensorE-row-stream bound, so the design minimizes total matmul
moving-rows (cost per matmul = out free size, regardless of contraction
or partition count):
  - host ships transposed hiddens qT/kT/vT [HID, S] fp16, the mask as
    48*(mT-1) in fp8 {0,-48}, and doubled weights [W|W] [HID, 2D] so one
    projection pass writes both PSUM partition groups 0-63 / 64-127
    (the row-duplicated layout the row-packed score matmuls need) in a
    single N-row stream.  Wq is pre-scaled by 1/sqrt(D).
  - projections on PE: per 512-col chunk, 8 contraction chunks,
    lhsT=[W|W][128,128] -> PSUM [128,512]; one DVE cast to f16
    QT/KT/VT [128, S] (rows 64-127 duplicate 0-63).
  - scores^T for k-tile pair (2j, 2j+1) land in one [128, 1024] PSUM
    tile via two row-packed f16 matmuls (contraction D=64 on partitions
    0-63 / 64-127).
  - mask is applied by DVE: pre = st + 48*(mT-1)  (PSUM f32 + fp8 ->
    f16 SBUF).  This frees the PSUM tile at DVE speed and costs the PE
    nothing.  exp runs on ScalarE (ACT) decoupled from the PE stream:
    masked entries become exp(s-48) which underflows f16 to exact 0.
  - out^T[65, q] += [V|1].T @ P^T accumulated over k-tiles: rows 0..63
    numerator, row 64 the softmax denominator.
  - norm: cast outT to f16, PE-transpose [65,128] slices back to
    [128,65], reciprocal of the [128,1] denominator column, multiply,
    DMA out [q, 64] as f32.
  - DMA: few large descriptors; q on SP (first), k/v on ACT queue
    (early, before exp starts), mask on gpsimd.  qh/kh live in a scoped
    pool that is reclaimed for the P tiles after the K projection.
"""

import numpy as np
import ml_dtypes

import concourse.bass as bass
import concourse.tile as tile
from concourse import bacc
from concourse import mybir
from concourse.bass_utils import run_bass_kernel_spmd

B, S, HID, D = 8, 2048, 1024, 64
NCORES = 8
HCH = HID // 128          # 8 hidden chunks
KT_TILES = S // 128       # 16 k tiles
NQ = 512                  # q chunk width for the attention inner loop
QCH = S // NQ             # 4
NPAIR = KT_TILES // 2     # 8 k-tile pairs
MASK_C = 48.0             # mask offset (48 exactly representable in e4m3)

F32 = mybir.dt.float32
F16 = mybir.dt.float16
FP8 = mybir.dt.float8e4
F16_NP = np.float16
FP8_NP = ml_dtypes.float8_e4m3

LAST_EXEC_TIME_NS = None
_CACHED = {}


def _build_program(with_qk_bias=False, reps=1):
    nc = bacc.Bacc("TRN2", target_bir_lowering=False, debug=False,
                   num_swdge_queues=4)

    qT_d = nc.dram_tensor("qT", [HID, S], F16, kind="ExternalInput").ap()
    kT_d = nc.dram_tensor("kT", [HID, S], F16, kind="ExternalInput").ap()
    vT_d = nc.dram_tensor("vT", [HID, S], F16, kind="ExternalInput").ap()
    # 48*(mask.T - 1): 0 where visible, -48 where masked
    maskT_d = nc.dram_tensor("maskT", [S, S], FP8, kind="ExternalInput").ap()
    # all three doubled weights [W | W], packed partition-major so one 2D
    # DMA ships them: wall[p, (j*8+h)*128 + d] = W2_j[h*128+p, d]
    wall_d = nc.dram_tensor("wall", [128, 3 * HCH * 128], F16,
                            kind="ExternalInput").ap()
    idm_d = nc.dram_tensor("idm", [128, 128], FP8, kind="ExternalInput").ap()
    if with_qk_bias:
        bq_d = nc.dram_tensor("bq", [D], F32, kind="ExternalInput").ap()
        bk_d = nc.dram_tensor("bk", [D], F32, kind="ExternalInput").ap()
    idf_d = nc.dram_tensor("idf", [128, 128], F16, kind="ExternalInput").ap()
    # [qc, p, t*D+d] layout so the output DMA is a clean 2D descriptor;
    # host untangles with a reshape/transpose.
    out_d = nc.dram_tensor("out", [QCH, 128, (NQ // 128) * D], F32,
                           kind="ExternalOutput").ap()

    ExpF = mybir.ActivationFunctionType.Exp

    def _body(tc):
        with tc.tile_pool(name="const", bufs=1) as const:
            w_all = const.tile([128, 3, HCH, 2 * D], F16, name="w_all")
            nc.sync.dma_start(w_all, wall_d.rearrange(
                "p (j o d) -> p j o d", j=3, o=HCH))
            w_q, w_k, w_v = (w_all[:, j] for j in range(3))
            idf16 = const.tile([128, 128], F16, name="idf16")
            idm = const.tile([128, 128], FP8, name="idm")
            if with_qk_bias:
                b_q = const.tile([128, 1], F32, name="b_q")
                b_k = const.tile([128, 1], F32, name="b_k")
                nc.sync.dma_start(b_q[0:D, :], bq_d.unsqueeze(1))
                nc.sync.dma_start(b_q[64:64 + D, :], bq_d.unsqueeze(1))
                nc.sync.dma_start(b_k[0:D, :], bk_d.unsqueeze(1))
                nc.sync.dma_start(b_k[64:64 + D, :], bk_d.unsqueeze(1))
            else:
                b_q = b_k = None

            masksb = const.tile([128, KT_TILES, S], FP8, name="masksb")
            vh = const.tile([128, HCH, S], F16, name="vh")
            QT = const.tile([128, S], F16, name="QT")
            KT = const.tile([128, S], F16, name="KT")
            VT = const.tile([128, S], F16, name="VT")
            Vt = const.tile([128, KT_TILES, D + 1], F16, name="Vt")

            # Each issuing engine owns one ~115 GB/s DMA queue (sync /
            # scalar hw queues, gpsimd swdge); aggregate ~330 GB/s.  Every
            # tensor is striped across all three queues, ordered so each
            # queue delivers: weights, q rows, k rows, mask tiles, v rows.
            ENG_H = {0: (0, 3, 6), 1: (1, 4, 7), 2: (2, 5)}
            ENG_MASK = {0: (0, 4, 6, 9, 12), 1: (1, 5, 7, 10, 13),
                        2: (2, 3, 8, 11, 14, 15)}

            def dma_rows(t, d, eng, hs):
                for h in hs:
                    eng.dma_start(t[:, h, :], d[h * 128:(h + 1) * 128, :])

            with tc.tile_pool(name="stp", bufs=2, space="PSUM") as stp:

                def proj_wide(hid_t, w_t, b_t, dest):
                    # h-outer over all 4 column chunks: the PE consumes
                    # each arriving [128, S] hidden row immediately.
                    prjs = [stp.tile([128, NQ], F32, name="prj", tag="st",
                                     bufs=6) for _ in range(QCH)]
                    for h in range(HCH):
                        for c in range(QCH):
                            nc.tensor.matmul(
                                prjs[c], lhsT=w_t[:, h, :],
                                rhs=hid_t[:, h, c * NQ:(c + 1) * NQ],
                                start=(h == 0), stop=(h == HCH - 1))
                    for c in range(QCH):
                        cs = slice(c * NQ, (c + 1) * NQ)
                        nc.vector.tensor_copy(dest[:, cs], prjs[c])
                        if b_t is not None:
                            nc.vector.tensor_scalar_add(
                                dest[:, cs], dest[:, cs], b_t)

                def proj(hid_t, w_t, b_t, dest, c, copy_eng):
                    cs = slice(c * NQ, (c + 1) * NQ)
                    prj = stp.tile([128, NQ], F32, name="prj", tag="prj",
                                   bufs=2)
                    for h in range(HCH):
                        nc.tensor.matmul(
                            prj, lhsT=w_t[:, h, :], rhs=hid_t[:, h, cs],
                            start=(h == 0), stop=(h == HCH - 1))
                    copy_eng.tensor_copy(dest[:, cs], prj)
                    if b_t is not None:
                        copy_eng.tensor_scalar_add(dest[:, cs], dest[:, cs],
                                                   b_t)

                # ---- staged emission (PE stream order == data arrival) --
                with tc.tile_pool(name="qkh", bufs=1) as qkh:
                    qh = qkh.tile([128, HCH, S], F16, name="qh")
                    kh = qkh.tile([128, HCH, S], F16, name="kh")

                    engs = [nc.sync, nc.scalar, nc.gpsimd]
                    # per-queue programs
                    nc.sync.dma_start(w_all[:, 0], wall_d[:, 0:1024]
                                      .rearrange("p (o d) -> p o d", o=HCH))
                    nc.scalar.dma_start(w_all[:, 1], wall_d[:, 1024:2048]
                                        .rearrange("p (o d) -> p o d",
                                                   o=HCH))
                    nc.gpsimd.dma_start(w_all[:, 2], wall_d[:, 2048:3072]
                                        .rearrange("p (o d) -> p o d",
                                                   o=HCH))
                    nc.gpsimd.dma_start(idm, idm_d)
                    nc.gpsimd.dma_start(idf16, idf_d)
                    for i, eng in enumerate(engs):
                        dma_rows(qh, qT_d, eng, ENG_H[i])
                    for i, eng in enumerate(engs):
                        dma_rows(kh, kT_d, eng, ENG_H[i])
                    for i, eng in enumerate(engs):
                        for kt in ENG_MASK[i]:
                            eng.dma_start(
                                masksb[:, kt, :],
                                maskT_d[kt * 128:(kt + 1) * 128, :])
                    for i, eng in enumerate(engs):
                        dma_rows(vh, vT_d, eng, ENG_H[i])

                    proj_wide(qh, w_q, b_q, QT)
                    proj_wide(kh, w_k, b_k, KT)

                with tc.tile_pool(name="pre", bufs=32) as prep, \
                     tc.tile_pool(name="ptp", bufs=56) as ptp, \
                     tc.tile_pool(name="nsb", bufs=2) as nsb:

                    def sc_unit(qc, p):
                        # row-packed score pair (2p, 2p+1), one PSUM bank
                        # per k-tile so the drain pipeline runs 5 deep.
                        # Half a: mask offsets accumulated on the PE (fp8
                        # idm @ moffs), drained by a direct ACT exp.
                        # Half b: drained by a DVE add of the mask
                        # offsets; its exp is deferred.  Drain load is
                        # split evenly across ACT and DVE every unit.
                        q0 = qc * NQ
                        qsl = slice(q0, q0 + NQ)
                        kta, ktb = 2 * p, 2 * p + 1
                        sa = slice(kta * 128, kta * 128 + 128)
                        sb = slice(ktb * 128, ktb * 128 + 128)
                        sta = stp.tile([128, NQ], F32, name="sta", tag="st",
                                       bufs=6)
                        nc.tensor.matmul(
                            sta, lhsT=KT[0:D, sa], rhs=QT[0:D, qsl],
                            start=True, stop=False)
                        nc.tensor.matmul(
                            sta, lhsT=idm, rhs=masksb[:, kta, qsl],
                            start=False, stop=True)
                        pta = ptp.tile([128, NQ], F16, name="pta", tag="pt")
                        nc.scalar.activation(pta, sta, ExpF)
                        stb = stp.tile([128, NQ], F32, name="stb", tag="st",
                                       bufs=6)
                        nc.tensor.matmul(
                            stb, lhsT=KT[64:64 + D, sb],
                            rhs=QT[64:64 + D, qsl], start=True, stop=True)
                        pre = prep.tile([128, NQ], F16, name="pre",
                                        tag="pre")
                        nc.vector.tensor_add(pre, stb, masksb[:, ktb, qsl])
                        return pta, pre

                    def v_fin(kt):
                        vtr = stp.tile([128, D], F16, name="vtr", tag="prj",
                                       bufs=2)
                        nc.tensor.transpose(
                            vtr, VT[0:D, kt * 128:(kt + 1) * 128],
                            idf16[0:D, 0:D])
                        nc.vector.tensor_copy(Vt[:, kt, :D], vtr)

                    def av(outT, p, pta, ptb):
                        nc.tensor.matmul(
                            outT, lhsT=Vt[:, 2 * p, :], rhs=pta,
                            start=(p == 0), stop=False)
                        nc.tensor.matmul(
                            outT, lhsT=Vt[:, 2 * p + 1, :], rhs=ptb,
                            start=False, stop=(p == NPAIR - 1))

                    def norm(qc, outT):
                        outT_sb = nsb.tile([D + 1, NQ], F16, name="outT_sb",
                                           tag="outT_sb")
                        nc.vector.tensor_copy(outT_sb, outT)
                        o_big = nsb.tile([128, (NQ // 128) * D], F32,
                                         name="o_big", tag="o_big")
                        for i in range(NQ // 128):
                            tr = stp.tile([128, D + 1], F16, name="tr",
                                          tag="prj", bufs=2)
                            nc.tensor.transpose(
                                tr, outT_sb[:, i * 128:(i + 1) * 128],
                                idf16[:D + 1, :D + 1])
                            rcp = nsb.tile([128, 1], F32, name="rcp",
                                           tag="rcp")
                            nc.vector.reciprocal(rcp, tr[:, D:D + 1])
                            nc.vector.tensor_scalar_mul(
                                o_big[:, i * D:(i + 1) * D], tr[:, :D], rcp)
                        nc.sync.dma_start(out_d[qc], o_big)

                    # ones column of Vt written once
                    nc.gpsimd.memset(Vt[:, :, D:D + 1], 1.0)

                    # k-pair-major unit order: mask tile 2p is first
                    # needed ~2.6us * p into the phase, so mask DMAs can
                    # trickle in behind the q/k rows.
                    pts = {}
                    pres = {}
                    for p in range(NPAIR):
                        for qc in range(QCH):
                            pts[(qc, p)], pres[(qc, p)] = sc_unit(qc, p)
                    for c in range(QCH):
                        proj(vh, w_v, None, VT, c, nc.vector)
                        for kt in range(4 * c, 4 * c + 4):
                            v_fin(kt)
                    # deferred exps for the b-halves (ACT, off the PE
                    # critical path)
                    ptbs = {}
                    for qc in range(QCH):
                        for p in range(NPAIR):
                            ptb = ptp.tile([128, NQ], F16, name="ptb",
                                           tag="pt")
                            nc.scalar.activation(ptb, pres[(qc, p)], ExpF)
                            ptbs[(qc, p)] = ptb
                    for qc in range(QCH):
                        outT = stp.tile([D + 1, NQ], F32, name="outT",
                                        tag="st", bufs=6)
                        for p in range(NPAIR):
                            av(outT, p, pts[(qc, p)], ptbs[(qc, p)])
                        norm(qc, outT)

    with tile.TileContext(nc) as tc:
        if reps > 1:
            with tc.For_i(0, reps, 1):
                _body(tc)
        else:
            _body(tc)

    nc.compile()
    return nc


def _prep_inputs(q_hidden_inputs, k_hidden_inputs, v_hidden_inputs, mask,
                 Wq, bq, Wk, bk, Wv, bv):
    scale = np.float32(1.0 / np.sqrt(np.float32(D)))
    wq = (np.asarray(Wq, np.float32) * scale).astype(F16_NP)
    wk = np.asarray(Wk, np.float32).astype(F16_NP)
    wv = np.asarray(Wv, np.float32).astype(F16_NP)
    wq2 = np.concatenate([wq, wq], axis=1)
    wk2 = np.concatenate([wk, wk], axis=1)
    wv2 = np.concatenate([wv, wv], axis=1)
    # wall[p, (j*8+h)*128 + d] = W2_j[h*128+p, d]
    wall = np.ascontiguousarray(
        np.stack([wq2, wk2, wv2])               # [3, HID, 2D]
        .reshape(3, HCH, 128, 2 * D)            # [3, h, p, d]
        .transpose(2, 0, 1, 3)                  # [p, 3, h, d]
        .reshape(128, 3 * HCH * 2 * D))
    bqs = (np.asarray(bq, np.float32) * scale)
    bks = np.asarray(bk, np.float32)
    with_qk_bias = bool(np.any(bqs != 0) or np.any(bks != 0))
    idf = np.eye(128, dtype=np.float32).astype(F16_NP)
    idm = np.eye(128, dtype=np.float32).astype(FP8_NP)

    q = np.asarray(q_hidden_inputs, np.float32)
    k = np.asarray(k_hidden_inputs, np.float32)
    v = np.asarray(v_hidden_inputs, np.float32)
    m = np.asarray(mask)

    in_maps = []
    for b in range(B):
        im = {
            "qT": np.ascontiguousarray(q[b].T).astype(F16_NP),
            "kT": np.ascontiguousarray(k[b].T).astype(F16_NP),
            "vT": np.ascontiguousarray(v[b].T).astype(F16_NP),
            "maskT": ((np.ascontiguousarray(m[b].T) - np.int32(1)) *
                      np.float32(MASK_C)).astype(FP8_NP),
            "wall": wall, "idm": idm,
            "idf": idf,
        }
        if with_qk_bias:
            im["bq"] = bqs
            im["bk"] = bks
        in_maps.append(im)
    return in_maps, with_qk_bias


def kernel(q_hidden_inputs, k_hidden_inputs, v_hidden_inputs, mask,
           Wq, bq, Wk, bk, Wv, bv, trace=False):
    global LAST_EXEC_TIME_NS
    in_maps, with_qk_bias = _prep_inputs(
        q_hidden_inputs, k_hidden_inputs, v_hidden_inputs,
        mask, Wq, bq, Wk, bk, Wv, bv)
    key = ("nc", with_qk_bias)
    if key not in _CACHED:
        _CACHED[key] = _build_program(with_qk_bias)
    nc = _CACHED[key]

    res = run_bass_kernel_spmd(nc, in_maps, list(range(NCORES)), trace=trace)
    LAST_EXEC_TIME_NS = res.exec_time_ns
    # out_d is [qc, p, t*D+d] with q = qc*512 + t*128 + p
    out = np.stack(
        [res.results[b]["out"].reshape(QCH, 128, NQ // 128, D)
         .transpose(0, 2, 1, 3).reshape(S, D) for b in range(B)], axis=0)
    # bv folds into the output exactly: softmax rows sum to 1, so
    # attn @ (V + 1 bv^T) = attn @ V + bv.
    out = out + np.asarray(bv, np.float32)[None, None, :]
    return out


# revision 22
# speedup vs baseline: 1.1891x; 1.0271x over previous
"""Trainium2 Bass kernel for masked single-head attention.

Reference computation (per batch b):
    Q = q_hidden[b] @ Wq + bq            # [S, D]
    K = k_hidden[b] @ Wk + bk            # [S, D]
    V = v_hidden[b] @ Wv + bv            # [S, D]
    S_qk = (Q @ K.T) / sqrt(D)           # [S, S]
    S_qk = where(mask[b]==0, -1e9, S_qk)
    out[b] = softmax(S_qk, -1) @ V       # [S, D]

Sharding: data-parallel over batch, one batch per NeuronCore (B == 8 cores).
No collectives.

Device-side dataflow (per core, S=2048, HID=1024, D=64).  The kernel is
purely TensorE-row-stream bound, so the design minimizes total matmul
moving-rows (cost per matmul = out free size, regardless of contraction
or partition count):
  - host ships transposed hiddens qT/kT/vT [HID, S] fp16, the mask as
    48*(mT-1) in fp8 {0,-48}, and doubled weights [W|W] [HID, 2D] so one
    projection pass writes both PSUM partition groups 0-63 / 64-127
    (the row-duplicated layout the row-packed score matmuls need) in a
    single N-row stream.  Wq is pre-scaled by 1/sqrt(D).
  - projections on PE: per 512-col chunk, 8 contraction chunks,
    lhsT=[W|W][128,128] -> PSUM [128,512]; one DVE cast to f16
    QT/KT/VT [128, S] (rows 64-127 duplicate 0-63).
  - scores^T for k-tile pair (2j, 2j+1) land in one [128, 1024] PSUM
    tile via two row-packed f16 matmuls (contraction D=64 on partitions
    0-63 / 64-127).
  - mask is applied by DVE: pre = st + 48*(mT-1)  (PSUM f32 + fp8 ->
    f16 SBUF).  This frees the PSUM tile at DVE speed and costs the PE
    nothing.  exp runs on ScalarE (ACT) decoupled from the PE stream:
    masked entries become exp(s-48) which underflows f16 to exact 0.
  - out^T[65, q] += [V|1].T @ P^T accumulated over k-tiles: rows 0..63
    numerator, row 64 the softmax denominator.
  - norm: cast outT to f16, PE-transpose [65,128] slices back to
    [128,65], reciprocal of the [128,1] denominator column, multiply,
    DMA out [q, 64] as f32.
  - DMA: few large descriptors; q on SP (first), k/v on ACT queue
    (early, before exp starts), mask on gpsimd.  qh/kh live in a scoped
    pool that is reclaimed for the P tiles after the K projection.
"""

import numpy as np
import ml_dtypes

import concourse.bass as bass
import concourse.tile as tile
from concourse import bacc
from concourse import mybir
from concourse.bass_utils import run_bass_kernel_spmd

B, S, HID, D = 8, 2048, 1024, 64
NCORES = 8
HCH = HID // 128          # 8 hidden chunks
KT_TILES = S // 128       # 16 k tiles
NQ = 512                  # q chunk width for the attention inner loop
QCH = S // NQ             # 4
NPAIR = KT_TILES // 2     # 8 k-tile pairs
MASK_C = 48.0             # mask offset (48 exactly representable in e4m3)

F32 = mybir.dt.float32
F16 = mybir.dt.float16
FP8 = mybir.dt.float8e4
F16_NP = np.float16
FP8_NP = ml_dtypes.float8_e4m3

LAST_EXEC_TIME_NS = None
_CACHED = {}


def _build_program(with_qk_bias=False, reps=1):
    nc = bacc.Bacc("TRN2", target_bir_lowering=False, debug=False,
                   num_swdge_queues=4)

    qT_d = nc.dram_tensor("qT", [HID, S], F16, kind="ExternalInput").ap()
    kT_d = nc.dram_tensor("kT", [HID, S], F16, kind="ExternalInput").ap()
    vT_d = nc.dram_tensor("vT", [HID, S], F16, kind="ExternalInput").ap()
    # 48*(mask.T - 1): 0 where visible, -48 where masked
    maskT_d = nc.dram_tensor("maskT", [S, S], FP8, kind="ExternalInput").ap()
    # all three doubled weights [W | W], packed partition-major so one 2D
    # DMA ships them: wall[p, (j*8+h)*128 + d] = W2_j[h*128+p, d]
    wall_d = nc.dram_tensor("wall", [128, 3 * HCH * 128], F16,
                            kind="ExternalInput").ap()
    idm_d = nc.dram_tensor("idm", [128, 128], FP8, kind="ExternalInput").ap()
    if with_qk_bias:
        bq_d = nc.dram_tensor("bq", [D], F32, kind="ExternalInput").ap()
        bk_d = nc.dram_tensor("bk", [D], F32, kind="ExternalInput").ap()
    idf_d = nc.dram_tensor("idf", [128, 128], F16, kind="ExternalInput").ap()
    # [qc, p, t*D+d] layout so the output DMA is a clean 2D descriptor;
    # host untangles with a reshape/transpose.
    out_d = nc.dram_tensor("out", [QCH, 128, (NQ // 128) * D], F32,
                           kind="ExternalOutput").ap()

    ExpF = mybir.ActivationFunctionType.Exp

    def _body(tc):
        with tc.tile_pool(name="const", bufs=1) as const:
            w_all = const.tile([128, 3, HCH, 2 * D], F16, name="w_all")
            nc.sync.dma_start(w_all, wall_d.rearrange(
                "p (j o d) -> p j o d", j=3, o=HCH))
            w_q, w_k, w_v = (w_all[:, j] for j in range(3))
            idf16 = const.tile([128, 128], F16, name="idf16")
            idm = const.tile([128, 128], FP8, name="idm")
            if with_qk_bias:
                b_q = const.tile([128, 1], F32, name="b_q")
                b_k = const.tile([128, 1], F32, name="b_k")
                nc.sync.dma_start(b_q[0:D, :], bq_d.unsqueeze(1))
                nc.sync.dma_start(b_q[64:64 + D, :], bq_d.unsqueeze(1))
                nc.sync.dma_start(b_k[0:D, :], bk_d.unsqueeze(1))
                nc.sync.dma_start(b_k[64:64 + D, :], bk_d.unsqueeze(1))
            else:
                b_q = b_k = None

            masksb = const.tile([128, KT_TILES, S], FP8, name="masksb")
            vh = const.tile([128, HCH, S], F16, name="vh")
            QT = const.tile([128, S], F16, name="QT")
            KT = const.tile([128, S], F16, name="KT")
            VT = const.tile([128, S], F16, name="VT")
            Vt = const.tile([128, KT_TILES, D + 1], F16, name="Vt")

            qh = const.tile([128, HCH, S], F16, name="qh")
            kh = const.tile([128, HCH, S], F16, name="kh")

            # pt halves alias onto qh/kh 1KB slots: those reads end with
            # the q/k projections exactly when the P tiles are born, and
            # the per-slice WAR tracking orders them without any pool
            # drain.  Unit u: a-half in qh slot u, b-half in kh slot u.
            def qslot(t, u):
                return t[:, u // 4, (u % 4) * NQ:(u % 4 + 1) * NQ]

            # Each issuing engine owns one ~115 GB/s DMA queue (sync /
            # scalar hw queues, gpsimd swdge); aggregate ~330 GB/s.
            # Stripe every tensor round-robin across the queues in
            # consumption order: wq, q (chunk 0 first), k chunks with the
            # mask tiles trickling between, v rows last.
            engs = [nc.sync, nc.scalar, nc.gpsimd]
            rr_state = [0]

            def issue(dst, src):
                engs[rr_state[0] % 3].dma_start(dst, src)
                rr_state[0] += 1

            def issue_mask(kt):
                issue(masksb[:, kt, :], maskT_d[kt * 128:(kt + 1) * 128, :])

            nc.sync.dma_start(w_all[:, 0], wall_d[:, 0:1024]
                              .rearrange("p (o d) -> p o d", o=HCH))
            nc.scalar.dma_start(w_all[:, 1], wall_d[:, 1024:2048]
                                .rearrange("p (o d) -> p o d", o=HCH))
            nc.gpsimd.dma_start(w_all[:, 2], wall_d[:, 2048:3072]
                                .rearrange("p (o d) -> p o d", o=HCH))
            nc.gpsimd.dma_start(idm, idm_d)
            nc.gpsimd.dma_start(idf16, idf_d)
            for h in range(HCH):
                issue(qh[:, h, 0:NQ], qT_d[h * 128:(h + 1) * 128, 0:NQ])
            for h in range(HCH):
                issue(qh[:, h, NQ:S], qT_d[h * 128:(h + 1) * 128, NQ:S])
            for h in range(HCH):
                issue(kh[:, h, 0:NQ], kT_d[h * 128:(h + 1) * 128, 0:NQ])
            issue_mask(0)
            issue_mask(1)
            for h in range(HCH):
                issue(kh[:, h, NQ:S], kT_d[h * 128:(h + 1) * 128, NQ:S])
                if h % 2 == 1 and h < 8:
                    for kt in (h, h + 1):
                        issue_mask(kt + 1)
            for kt in range(10, KT_TILES):
                issue_mask(kt)
            for h in range(HCH):
                issue(vh[:, h, :], vT_d[h * 128:(h + 1) * 128, :])

            with tc.tile_pool(name="stp", bufs=2, space="PSUM") as stp, \
                 tc.tile_pool(name="nsb", bufs=2) as nsb:

                def proj(hid_t, w_t, b_t, dest, c, copy_eng):
                    cs = slice(c * NQ, (c + 1) * NQ)
                    prj = stp.tile([128, NQ], F32, name="prj", tag="prj",
                                   bufs=2)
                    for h in range(HCH):
                        nc.tensor.matmul(
                            prj, lhsT=w_t[:, h, :], rhs=hid_t[:, h, cs],
                            start=(h == 0), stop=(h == HCH - 1))
                    copy_eng.tensor_copy(dest[:, cs], prj)
                    if b_t is not None:
                        copy_eng.tensor_scalar_add(dest[:, cs], dest[:, cs],
                                                   b_t)

                # ---- staged emission (PE stream order == data arrival) --
                for c in range(QCH):
                    proj(qh, w_q, b_q, QT, c, nc.vector)
                for c in range(QCH):
                    proj(kh, w_k, b_k, KT, c, nc.vector)

                if True:

                    def sc_unit(qc, p):
                        # row-packed score pair (2p, 2p+1), one PSUM bank
                        # per k-tile so the drain pipeline runs deep.
                        # Half a: mask offsets accumulated on the PE (fp8
                        # idm @ moffs), drained by a direct ACT exp.
                        # Half b: drained by a DVE add of the mask
                        # offsets; its exp runs in-place, deferred.
                        u = 4 * p + qc
                        q0 = qc * NQ
                        qsl = slice(q0, q0 + NQ)
                        kta, ktb = 2 * p, 2 * p + 1
                        sa = slice(kta * 128, kta * 128 + 128)
                        sb = slice(ktb * 128, ktb * 128 + 128)
                        sta = stp.tile([128, NQ], F32, name="sta", tag="st",
                                       bufs=6)
                        nc.tensor.matmul(
                            sta, lhsT=KT[0:D, sa], rhs=QT[0:D, qsl],
                            start=True, stop=False)
                        nc.tensor.matmul(
                            sta, lhsT=idm, rhs=masksb[:, kta, qsl],
                            start=False, stop=True)
                        pta = qslot(qh, u)
                        nc.scalar.activation(pta, sta, ExpF)
                        stb = stp.tile([128, NQ], F32, name="stb", tag="st",
                                       bufs=6)
                        nc.tensor.matmul(
                            stb, lhsT=KT[64:64 + D, sb],
                            rhs=QT[64:64 + D, qsl], start=True, stop=True)
                        pre = qslot(kh, u)
                        nc.vector.tensor_add(pre, stb, masksb[:, ktb, qsl])
                        return pta, pre

                    def v_fin(kt):
                        vtr = stp.tile([128, D], F16, name="vtr", tag="prj",
                                       bufs=2)
                        nc.tensor.transpose(
                            vtr, VT[0:D, kt * 128:(kt + 1) * 128],
                            idf16[0:D, 0:D])
                        nc.vector.tensor_copy(Vt[:, kt, :D], vtr)

                    def av(outT, p, pta, ptb):
                        nc.tensor.matmul(
                            outT, lhsT=Vt[:, 2 * p, :], rhs=pta,
                            start=(p == 0), stop=False)
                        nc.tensor.matmul(
                            outT, lhsT=Vt[:, 2 * p + 1, :], rhs=ptb,
                            start=False, stop=(p == NPAIR - 1))

                    def norm(qc, outT):
                        outT_sb = nsb.tile([D + 1, NQ], F16, name="outT_sb",
                                           tag="outT_sb")
                        nc.vector.tensor_copy(outT_sb, outT)
                        o_big = nsb.tile([128, (NQ // 128) * D], F32,
                                         name="o_big", tag="o_big")
                        for i in range(NQ // 128):
                            tr = stp.tile([128, D + 1], F16, name="tr",
                                          tag="prj", bufs=2)
                            nc.tensor.transpose(
                                tr, outT_sb[:, i * 128:(i + 1) * 128],
                                idf16[:D + 1, :D + 1])
                            rcp = nsb.tile([128, 1], F32, name="rcp",
                                           tag="rcp")
                            nc.vector.reciprocal(rcp, tr[:, D:D + 1])
                            nc.vector.tensor_scalar_mul(
                                o_big[:, i * D:(i + 1) * D], tr[:, :D], rcp)
                        nc.sync.dma_start(out_d[qc], o_big)

                    # ones column of Vt written once
                    nc.gpsimd.memset(Vt[:, :, D:D + 1], 1.0)

                    # k-pair-major unit order: mask tile 2p is first
                    # needed ~2.6us * p into the phase, so mask DMAs can
                    # trickle in behind the q/k rows.
                    pts = {}
                    pres = {}
                    for p in range(NPAIR):
                        for qc in range(QCH):
                            pts[(qc, p)], pres[(qc, p)] = sc_unit(qc, p)
                    for c in range(QCH):
                        proj(vh, w_v, None, VT, c, nc.vector)
                        for kt in range(4 * c, 4 * c + 4):
                            v_fin(kt)
                    # deferred in-place exps for the b-halves (ACT, off
                    # the PE critical path)
                    for qc in range(QCH):
                        for p in range(NPAIR):
                            nc.scalar.activation(pres[(qc, p)],
                                                 pres[(qc, p)], ExpF)
                    for qc in range(QCH):
                        outT = stp.tile([D + 1, NQ], F32, name="outT",
                                        tag="st", bufs=6)
                        for p in range(NPAIR):
                            av(outT, p, pts[(qc, p)], pres[(qc, p)])
                        norm(qc, outT)

    with tile.TileContext(nc) as tc:
        if reps > 1:
            with tc.For_i(0, reps, 1):
                _body(tc)
        else:
            _body(tc)

    nc.compile()
    return nc


def _prep_inputs(q_hidden_inputs, k_hidden_inputs, v_hidden_inputs, mask,
                 Wq, bq, Wk, bk, Wv, bv):
    scale = np.float32(1.0 / np.sqrt(np.float32(D)))
    wq = (np.asarray(Wq, np.float32) * scale).astype(F16_NP)
    wk = np.asarray(Wk, np.float32).astype(F16_NP)
    wv = np.asarray(Wv, np.float32).astype(F16_NP)
    wq2 = np.concatenate([wq, wq], axis=1)
    wk2 = np.concatenate([wk, wk], axis=1)
    wv2 = np.concatenate([wv, wv], axis=1)
    # wall[p, (j*8+h)*128 + d] = W2_j[h*128+p, d]
    wall = np.ascontiguousarray(
        np.stack([wq2, wk2, wv2])               # [3, HID, 2D]
        .reshape(3, HCH, 128, 2 * D)            # [3, h, p, d]
        .transpose(2, 0, 1, 3)                  # [p, 3, h, d]
        .reshape(128, 3 * HCH * 2 * D))
    bqs = (np.asarray(bq, np.float32) * scale)
    bks = np.asarray(bk, np.float32)
    with_qk_bias = bool(np.any(bqs != 0) or np.any(bks != 0))
    idf = np.eye(128, dtype=np.float32).astype(F16_NP)
    idm = np.eye(128, dtype=np.float32).astype(FP8_NP)

    q = np.asarray(q_hidden_inputs, np.float32)
    k = np.asarray(k_hidden_inputs, np.float32)
    v = np.asarray(v_hidden_inputs, np.float32)
    m = np.asarray(mask)

    in_maps = []
    for b in range(B):
        im = {
            "qT": np.ascontiguousarray(q[b].T).astype(F16_NP),
            "kT": np.ascontiguousarray(k[b].T).astype(F16_NP),
            "vT": np.ascontiguousarray(v[b].T).astype(F16_NP),
            "maskT": ((np.ascontiguousarray(m[b].T) - np.int32(1)) *
                      np.float32(MASK_C)).astype(FP8_NP),
            "wall": wall, "idm": idm,
            "idf": idf,
        }
        if with_qk_bias:
            im["bq"] = bqs
            im["bk"] = bks
        in_maps.append(im)
    return in_maps, with_qk_bias


def kernel(q_hidden_inputs, k_hidden_inputs, v_hidden_inputs, mask,
           Wq, bq, Wk, bk, Wv, bv, trace=False):
    global LAST_EXEC_TIME_NS
    in_maps, with_qk_bias = _prep_inputs(
        q_hidden_inputs, k_hidden_inputs, v_hidden_inputs,
        mask, Wq, bq, Wk, bk, Wv, bv)
    key = ("nc", with_qk_bias)
    if key not in _CACHED:
        _CACHED[key] = _build_program(with_qk_bias)
    nc = _CACHED[key]

    res = run_bass_kernel_spmd(nc, in_maps, list(range(NCORES)), trace=trace)
    LAST_EXEC_TIME_NS = res.exec_time_ns
    # out_d is [qc, p, t*D+d] with q = qc*512 + t*128 + p
    out = np.stack(
        [res.results[b]["out"].reshape(QCH, 128, NQ // 128, D)
         .transpose(0, 2, 1, 3).reshape(S, D) for b in range(B)], axis=0)
    # bv folds into the output exactly: softmax rows sum to 1, so
    # attn @ (V + 1 bv^T) = attn @ V + bv.
    out = out + np.asarray(bv, np.float32)[None, None, :]
    return out


# revision 24
# speedup vs baseline: 1.2501x; 1.0513x over previous
"""Trainium2 Bass kernel for masked single-head attention.

Reference computation (per batch b):
    Q = q_hidden[b] @ Wq + bq            # [S, D]
    K = k_hidden[b] @ Wk + bk            # [S, D]
    V = v_hidden[b] @ Wv + bv            # [S, D]
    S_qk = (Q @ K.T) / sqrt(D)           # [S, S]
    S_qk = where(mask[b]==0, -1e9, S_qk)
    out[b] = softmax(S_qk, -1) @ V       # [S, D]

Sharding: data-parallel over batch, one batch per NeuronCore (B == 8 cores).
No collectives.

Device-side dataflow (per core, S=2048, HID=1024, D=64).  The kernel is
purely TensorE-row-stream bound, so the design minimizes total matmul
moving-rows (cost per matmul = out free size, regardless of contraction
or partition count):
  - host ships transposed hiddens qT/kT/vT [HID, S] fp16, the mask as
    48*(mT-1) in fp8 {0,-48}, and doubled weights [W|W] [HID, 2D] so one
    projection pass writes both PSUM partition groups 0-63 / 64-127
    (the row-duplicated layout the row-packed score matmuls need) in a
    single N-row stream.  Wq is pre-scaled by 1/sqrt(D).
  - projections on PE: per 512-col chunk, 8 contraction chunks,
    lhsT=[W|W][128,128] -> PSUM [128,512]; one DVE cast to f16
    QT/KT/VT [128, S] (rows 64-127 duplicate 0-63).
  - scores^T for k-tile pair (2j, 2j+1) land in one [128, 1024] PSUM
    tile via two row-packed f16 matmuls (contraction D=64 on partitions
    0-63 / 64-127).
  - mask is applied by DVE: pre = st + 48*(mT-1)  (PSUM f32 + fp8 ->
    f16 SBUF).  This frees the PSUM tile at DVE speed and costs the PE
    nothing.  exp runs on ScalarE (ACT) decoupled from the PE stream:
    masked entries become exp(s-48) which underflows f16 to exact 0.
  - out^T[65, q] += [V|1].T @ P^T accumulated over k-tiles: rows 0..63
    numerator, row 64 the softmax denominator.
  - norm: cast outT to f16, PE-transpose [65,128] slices back to
    [128,65], reciprocal of the [128,1] denominator column, multiply,
    DMA out [q, 64] as f32.
  - DMA: few large descriptors; q on SP (first), k/v on ACT queue
    (early, before exp starts), mask on gpsimd.  qh/kh live in a scoped
    pool that is reclaimed for the P tiles after the K projection.
"""

import numpy as np
import ml_dtypes

import concourse.bass as bass
import concourse.tile as tile
from concourse import bacc
from concourse import mybir
from concourse.bass_utils import run_bass_kernel_spmd

B, S, HID, D = 8, 2048, 1024, 64
NCORES = 8
HCH = HID // 128          # 8 hidden chunks
KT_TILES = S // 128       # 16 k tiles
NQ = 512                  # q chunk width for the attention inner loop
QCH = S // NQ             # 4
NPAIR = KT_TILES // 2     # 8 k-tile pairs
MASK_C = 48.0             # mask offset (48 exactly representable in e4m3)

F32 = mybir.dt.float32
F16 = mybir.dt.float16
FP8 = mybir.dt.float8e4
F16_NP = np.float16
FP8_NP = ml_dtypes.float8_e4m3

LAST_EXEC_TIME_NS = None
_CACHED = {}


def _build_program(with_qk_bias=False, reps=1):
    nc = bacc.Bacc("TRN2", target_bir_lowering=False, debug=False,
                   num_swdge_queues=4)

    qT_d = nc.dram_tensor("qT", [HID, S], F16, kind="ExternalInput").ap()
    kT_d = nc.dram_tensor("kT", [HID, S], F16, kind="ExternalInput").ap()
    vT_d = nc.dram_tensor("vT", [HID, S], F16, kind="ExternalInput").ap()
    # 48*(mask.T - 1): 0 where visible, -48 where masked
    maskT_d = nc.dram_tensor("maskT", [S, S], FP8, kind="ExternalInput").ap()
    # all three doubled weights [W | W], packed partition-major so one 2D
    # DMA ships them: wall[p, (j*8+h)*128 + d] = W2_j[h*128+p, d]
    wall_d = nc.dram_tensor("wall", [128, 3 * HCH * 128], F16,
                            kind="ExternalInput").ap()
    idm_d = nc.dram_tensor("idm", [128, 128], FP8, kind="ExternalInput").ap()
    if with_qk_bias:
        bq_d = nc.dram_tensor("bq", [D], F32, kind="ExternalInput").ap()
        bk_d = nc.dram_tensor("bk", [D], F32, kind="ExternalInput").ap()
    idf_d = nc.dram_tensor("idf", [128, 128], F16, kind="ExternalInput").ap()
    # [qc, p, t*D+d] layout so the output DMA is a clean 2D descriptor;
    # host untangles with a reshape/transpose.
    out_d = nc.dram_tensor("out", [QCH, 128, (NQ // 128) * D], F32,
                           kind="ExternalOutput").ap()

    ExpF = mybir.ActivationFunctionType.Exp

    def _body(tc):
        with tc.tile_pool(name="const", bufs=1) as const:
            w_all = const.tile([128, 3, HCH, 2 * D], F16, name="w_all")
            w_q, w_k, w_v = (w_all[:, j] for j in range(3))
            idf16 = const.tile([128, 128], F16, name="idf16")
            idm = const.tile([128, 128], FP8, name="idm")
            if with_qk_bias:
                b_q = const.tile([128, 1], F32, name="b_q")
                b_k = const.tile([128, 1], F32, name="b_k")
                nc.sync.dma_start(b_q[0:D, :], bq_d.unsqueeze(1))
                nc.sync.dma_start(b_q[64:64 + D, :], bq_d.unsqueeze(1))
                nc.sync.dma_start(b_k[0:D, :], bk_d.unsqueeze(1))
                nc.sync.dma_start(b_k[64:64 + D, :], bk_d.unsqueeze(1))
            else:
                b_q = b_k = None

            masksb = const.tile([128, KT_TILES, S], FP8, name="masksb")
            vh = const.tile([128, HCH, S], F16, name="vh")
            QT = const.tile([128, S], F16, name="QT")
            KT = const.tile([128, S], F16, name="KT")
            VT = const.tile([128, S], F16, name="VT")
            Vt = const.tile([128, KT_TILES, D + 1], F16, name="Vt")

            qh = const.tile([128, HCH, S], F16, name="qh")
            kh = const.tile([128, HCH, S], F16, name="kh")

            # pt halves alias onto qh/kh 1KB slots: those reads end with
            # the q/k projections exactly when the P tiles are born, and
            # the per-slice WAR tracking orders them without any pool
            # drain.  Unit u: a-half in qh slot u, b-half in kh slot u.
            def qslot(t, u):
                return t[:, u // 4, (u % 4) * NQ:(u % 4 + 1) * NQ]

            # Each issuing engine owns one ~115 GB/s DMA queue (sync /
            # scalar hw queues, gpsimd swdge); aggregate ~330 GB/s.
            # Stripe every tensor round-robin across the queues in
            # consumption order: wq, q (chunk 0 first), k chunks with the
            # mask tiles trickling between, v rows last.
            engs = [nc.sync, nc.scalar, nc.gpsimd]
            rr_state = [0]

            def issue(dst, src):
                engs[rr_state[0] % 3].dma_start(dst, src)
                rr_state[0] += 1

            def issue_mask(kt):
                issue(masksb[:, kt, :], maskT_d[kt * 128:(kt + 1) * 128, :])

            nc.sync.dma_start(w_all[:, 0], wall_d[:, 0:1024]
                              .rearrange("p (o d) -> p o d", o=HCH))
            nc.scalar.dma_start(w_all[:, 1], wall_d[:, 1024:2048]
                                .rearrange("p (o d) -> p o d", o=HCH))
            nc.gpsimd.dma_start(w_all[:, 2], wall_d[:, 2048:3072]
                                .rearrange("p (o d) -> p o d", o=HCH))
            nc.gpsimd.dma_start(idm, idm_d)
            nc.gpsimd.dma_start(idf16, idf_d)
            for h in range(HCH):
                issue(qh[:, h, 0:NQ], qT_d[h * 128:(h + 1) * 128, 0:NQ])

            with tc.tile_pool(name="stp", bufs=2, space="PSUM") as stp, \
                 tc.tile_pool(name="nsb", bufs=2) as nsb:

                def proj(hid_t, w_t, b_t, dest, c, copy_eng):
                    cs = slice(c * NQ, (c + 1) * NQ)
                    prj = stp.tile([128, NQ], F32, name="prj", tag="prj",
                                   bufs=2)
                    for h in range(HCH):
                        nc.tensor.matmul(
                            prj, lhsT=w_t[:, h, :], rhs=hid_t[:, h, cs],
                            start=(h == 0), stop=(h == HCH - 1))
                    copy_eng.tensor_copy(dest[:, cs], prj)
                    if b_t is not None:
                        copy_eng.tensor_scalar_add(dest[:, cs], dest[:, cs],
                                                   b_t)

                # ---- staged emission, DMA issues interleaved so the
                # scheduler's merged waits stay tight ----
                proj(qh, w_q, b_q, QT, 0, nc.vector)
                for h in range(HCH):
                    issue(qh[:, h, NQ:S], qT_d[h * 128:(h + 1) * 128, NQ:S])
                for c in range(1, QCH):
                    proj(qh, w_q, b_q, QT, c, nc.vector)
                for h in range(HCH):
                    issue(kh[:, h, 0:NQ], kT_d[h * 128:(h + 1) * 128, 0:NQ])
                for h in range(HCH):
                    issue(kh[:, h, NQ:S], kT_d[h * 128:(h + 1) * 128, NQ:S])
                for c in range(QCH):
                    proj(kh, w_k, b_k, KT, c, nc.vector)
                for h in range(HCH):
                    issue(vh[:, h, :], vT_d[h * 128:(h + 1) * 128, :])

                if True:

                    def sc_unit(qc, p):
                        # row-packed score pair (2p, 2p+1), one PSUM bank
                        # per k-tile so the drain pipeline runs deep.
                        # Half a: mask offsets accumulated on the PE (fp8
                        # idm @ moffs), drained by a direct ACT exp.
                        # Half b: drained by a DVE add of the mask
                        # offsets; its exp runs in-place, deferred.
                        u = 4 * p + qc
                        q0 = qc * NQ
                        qsl = slice(q0, q0 + NQ)
                        kta, ktb = 2 * p, 2 * p + 1
                        sa = slice(kta * 128, kta * 128 + 128)
                        sb = slice(ktb * 128, ktb * 128 + 128)
                        sta = stp.tile([128, NQ], F32, name="sta", tag="st",
                                       bufs=6)
                        nc.tensor.matmul(
                            sta, lhsT=KT[0:D, sa], rhs=QT[0:D, qsl],
                            start=True, stop=False)
                        nc.tensor.matmul(
                            sta, lhsT=idm, rhs=masksb[:, kta, qsl],
                            start=False, stop=True)
                        pta = qslot(qh, u)
                        nc.scalar.activation(pta, sta, ExpF)
                        stb = stp.tile([128, NQ], F32, name="stb", tag="st",
                                       bufs=6)
                        nc.tensor.matmul(
                            stb, lhsT=KT[64:64 + D, sb],
                            rhs=QT[64:64 + D, qsl], start=True, stop=True)
                        pre = qslot(kh, u)
                        nc.vector.tensor_add(pre, stb, masksb[:, ktb, qsl])
                        return pta, pre

                    def v_fin(kt):
                        vtr = stp.tile([128, D], F16, name="vtr", tag="prj",
                                       bufs=2)
                        nc.tensor.transpose(
                            vtr, VT[0:D, kt * 128:(kt + 1) * 128],
                            idf16[0:D, 0:D])
                        nc.vector.tensor_copy(Vt[:, kt, :D], vtr)

                    def av(outT, p, pta, ptb):
                        nc.tensor.matmul(
                            outT, lhsT=Vt[:, 2 * p, :], rhs=pta,
                            start=(p == 0), stop=False)
                        nc.tensor.matmul(
                            outT, lhsT=Vt[:, 2 * p + 1, :], rhs=ptb,
                            start=False, stop=(p == NPAIR - 1))

                    def norm(qc, outT):
                        outT_sb = nsb.tile([D + 1, NQ], F16, name="outT_sb",
                                           tag="outT_sb")
                        nc.vector.tensor_copy(outT_sb, outT)
                        o_big = nsb.tile([128, (NQ // 128) * D], F32,
                                         name="o_big", tag="o_big")
                        for i in range(NQ // 128):
                            tr = stp.tile([128, D + 1], F16, name="tr",
                                          tag="prj", bufs=2)
                            nc.tensor.transpose(
                                tr, outT_sb[:, i * 128:(i + 1) * 128],
                                idf16[:D + 1, :D + 1])
                            rcp = nsb.tile([128, 1], F32, name="rcp",
                                           tag="rcp")
                            nc.vector.reciprocal(rcp, tr[:, D:D + 1])
                            nc.vector.tensor_scalar_mul(
                                o_big[:, i * D:(i + 1) * D], tr[:, :D], rcp)
                        nc.sync.dma_start(out_d[qc], o_big)

                    # ones column of Vt written once
                    nc.gpsimd.memset(Vt[:, :, D:D + 1], 1.0)

                    # k-pair-major unit order: mask tile 2p is first
                    # needed ~2.6us * p into the phase, so mask DMAs can
                    # trickle in behind the q/k rows.
                    pts = {}
                    pres = {}
                    for p in range(NPAIR):
                        issue_mask(2 * p)
                        issue_mask(2 * p + 1)
                        for qc in range(QCH):
                            pts[(qc, p)], pres[(qc, p)] = sc_unit(qc, p)
                    for c in range(QCH):
                        proj(vh, w_v, None, VT, c, nc.vector)
                        for kt in range(4 * c, 4 * c + 4):
                            v_fin(kt)
                    # deferred in-place exps for the b-halves (ACT, off
                    # the PE critical path)
                    for qc in range(QCH):
                        for p in range(NPAIR):
                            nc.scalar.activation(pres[(qc, p)],
                                                 pres[(qc, p)], ExpF)
                    for qc in range(QCH):
                        outT = stp.tile([D + 1, NQ], F32, name="outT",
                                        tag="st", bufs=6)
                        for p in range(NPAIR):
                            av(outT, p, pts[(qc, p)], pres[(qc, p)])
                        norm(qc, outT)

    with tile.TileContext(nc) as tc:
        if reps > 1:
            with tc.For_i(0, reps, 1):
                _body(tc)
        else:
            _body(tc)

    nc.compile()
    return nc


def _prep_inputs(q_hidden_inputs, k_hidden_inputs, v_hidden_inputs, mask,
                 Wq, bq, Wk, bk, Wv, bv):
    scale = np.float32(1.0 / np.sqrt(np.float32(D)))
    wq = (np.asarray(Wq, np.float32) * scale).astype(F16_NP)
    wk = np.asarray(Wk, np.float32).astype(F16_NP)
    wv = np.asarray(Wv, np.float32).astype(F16_NP)
    wq2 = np.concatenate([wq, wq], axis=1)
    wk2 = np.concatenate([wk, wk], axis=1)
    wv2 = np.concatenate([wv, wv], axis=1)
    # wall[p, (j*8+h)*128 + d] = W2_j[h*128+p, d]
    wall = np.ascontiguousarray(
        np.stack([wq2, wk2, wv2])               # [3, HID, 2D]
        .reshape(3, HCH, 128, 2 * D)            # [3, h, p, d]
        .transpose(2, 0, 1, 3)                  # [p, 3, h, d]
        .reshape(128, 3 * HCH * 2 * D))
    bqs = (np.asarray(bq, np.float32) * scale)
    bks = np.asarray(bk, np.float32)
    with_qk_bias = bool(np.any(bqs != 0) or np.any(bks != 0))
    idf = np.eye(128, dtype=np.float32).astype(F16_NP)
    idm = np.eye(128, dtype=np.float32).astype(FP8_NP)

    q = np.asarray(q_hidden_inputs, np.float32)
    k = np.asarray(k_hidden_inputs, np.float32)
    v = np.asarray(v_hidden_inputs, np.float32)
    m = np.asarray(mask)

    in_maps = []
    for b in range(B):
        im = {
            "qT": np.ascontiguousarray(q[b].T).astype(F16_NP),
            "kT": np.ascontiguousarray(k[b].T).astype(F16_NP),
            "vT": np.ascontiguousarray(v[b].T).astype(F16_NP),
            "maskT": ((np.ascontiguousarray(m[b].T) - np.int32(1)) *
                      np.float32(MASK_C)).astype(FP8_NP),
            "wall": wall, "idm": idm,
            "idf": idf,
        }
        if with_qk_bias:
            im["bq"] = bqs
            im["bk"] = bks
        in_maps.append(im)
    return in_maps, with_qk_bias


def kernel(q_hidden_inputs, k_hidden_inputs, v_hidden_inputs, mask,
           Wq, bq, Wk, bk, Wv, bv, trace=False):
    global LAST_EXEC_TIME_NS
    in_maps, with_qk_bias = _prep_inputs(
        q_hidden_inputs, k_hidden_inputs, v_hidden_inputs,
        mask, Wq, bq, Wk, bk, Wv, bv)
    key = ("nc", with_qk_bias)
    if key not in _CACHED:
        _CACHED[key] = _build_program(with_qk_bias)
    nc = _CACHED[key]

    res = run_bass_kernel_spmd(nc, in_maps, list(range(NCORES)), trace=trace)
    LAST_EXEC_TIME_NS = res.exec_time_ns
    # out_d is [qc, p, t*D+d] with q = qc*512 + t*128 + p
    out = np.stack(
        [res.results[b]["out"].reshape(QCH, 128, NQ // 128, D)
         .transpose(0, 2, 1, 3).reshape(S, D) for b in range(B)], axis=0)
    # bv folds into the output exactly: softmax rows sum to 1, so
    # attn @ (V + 1 bv^T) = attn @ V + bv.
    out = out + np.asarray(bv, np.float32)[None, None, :]
    return out
